# revision 1
# baseline (speedup 1.0000x reference)
"""Multi-head attention (16 heads, D=1024, B=2, S=2048) on 8 TRN2 NeuronCores.

Sharding: tensor-parallel over heads. Each core owns 2 heads (128 features):
W_q/k/v column-sliced, W_o row-sliced; partial outputs summed on host.

Device dataflow (per core), everything kept "transposed" (features on
partitions) so the key-padding mask folds into the ACT exp bias and the
attention matrix is produced directly in the layout the A@V matmul needs:

  QT[f,s] = Wq_c^T @ x^T        (PE, contraction d on partitions)
  KT[f,s] likewise; VT -> PE-transpose -> V[s,f] (natural, k on partitions)
  scores^T[k,q] = KT_h^T-slice . QT_h  (row-tiled pairs, 2 heads)
  attn^T = exp(scores*0.125 + mask_bias[k])   (ACT, PSUM->SBUF, bf16)
  out_h^T[d,q] (+rowsum in row 64) = [V_h | 1]^T . attn_h^T  (PSUM accum over k)
  normalize: recip(rowsum) broadcast over partitions via tiny PE matmul
  out_part[s,:] = outT^T . Wo_c  -> fp16 partial, host sums in fp32

Key-padding mask: k-chunks entirely beyond valid_len are skipped (program is
specialized to the valid_lens values at call time); the boundary chunk uses a
-1e6 additive bias inside the exp activation (exp underflows to exact 0).
"""

import math
import os

import ml_dtypes
import numpy as np

B = 2
S = 2048
D = 1024
NT = B * S          # 4096 rows, b-major
F = 128             # features per core (2 heads x 64)
DH = 64
P = 128
DK = D // P         # 8 contraction chunks for projections
N_CORES = 8
NEG = -1e6

_CACHE: dict = {}


def _build_program(KC: tuple[int, int], cfg: dict):
    import concourse.bass as bass
    import concourse.tile as tile
    from concourse import mybir
    from concourse.masks import make_identity

    dt = mybir.dt
    DT_IN = getattr(dt, cfg["dt_in"])        # xT + W in HBM / matmul dtype
    DT_ATTN = getattr(dt, cfg["dt_attn"])    # attn / V / QT / KT storage
    DT_OUT = getattr(dt, cfg["dt_out"])      # partial output in HBM

    nc = bass.Bass("TRN2")

    xtq_d = nc.dram_tensor("xtq", [D, NT], DT_IN, kind="ExternalInput")
    xtk_d = nc.dram_tensor("xtk", [D, NT], DT_IN, kind="ExternalInput")
    xtv_d = nc.dram_tensor("xtv", [D, NT], DT_IN, kind="ExternalInput")
    wq_d = nc.dram_tensor("wq", [D, F], DT_IN, kind="ExternalInput")
    wk_d = nc.dram_tensor("wk", [D, F], DT_IN, kind="ExternalInput")
    wv_d = nc.dram_tensor("wv", [D, F], DT_IN, kind="ExternalInput")
    wo_d = nc.dram_tensor("wo", [F, D], DT_IN, kind="ExternalInput")
    mask_d = nc.dram_tensor("maskt", [P, B * 16], dt.float32, kind="ExternalInput")
    out_d = nc.dram_tensor("out_part", [NT, D], DT_OUT, kind="ExternalOutput")

    # global 512-wide seq tiles needed for K/V per batch (only up to valid_len)
    ntk = []
    for b in range(B):
        for t in range(math.ceil(KC[b] * 128 / 512)):
            ntk.append(b * 4 + t)
    NQT = NT // 512  # 8 q tiles of 512

    from contextlib import ExitStack

    B_X = int(cfg.get("b_x", 3))        # x streaming tiles per kc tag
    B_AT = int(cfg.get("b_at", 4))      # attn tiles
    B_SC = int(cfg.get("b_sc", 2))      # score psum banks
    B_AV = int(cfg.get("b_av", 2))      # av psum banks
    B_PROJ = int(cfg.get("b_proj", 1))  # proj psum banks
    B_PW = int(cfg.get("b_pw", 1))      # Wo psum banks
    assert 2 * B_SC + B_AV + B_PROJ + B_PW <= 8

    with tile.TileContext(nc) as tc, ExitStack() as ctx:
        const = ctx.enter_context(tc.tile_pool(name="const", bufs=1))
        xpool = ctx.enter_context(tc.tile_pool(name="xpool", bufs=B_X))
        apool = ctx.enter_context(tc.tile_pool(name="apool", bufs=B_AT))
        rpool = ctx.enter_context(tc.tile_pool(name="rpool", bufs=2))
        ps_score = ctx.enter_context(
            tc.tile_pool(name="ps_score", bufs=B_SC, space="PSUM"))
        ps_av = ctx.enter_context(
            tc.tile_pool(name="ps_av", bufs=B_AV, space="PSUM"))
        ps_misc = ctx.enter_context(
            tc.tile_pool(name="ps_misc", bufs=2, space="PSUM"))

        # ---- constants ----
        # Matmult instructions tolerate only ONE sync-wait, so every tensor a
        # matmul reads must be written by DVE (one mergeable semaphore): all
        # weight/identity loads bounce DRAM -> raw tile -> DVE copy -> tile.
        def dve_load(dst, src_ap, raw_shape, raw_dtype, nm):
            raw = const.tile(list(raw_shape), raw_dtype, tag=f"{nm}_raw",
                             name=f"{nm}_raw")
            nc.sync.dma_start(raw, src_ap)
            nc.vector.tensor_copy(out=dst, in_=raw)

        wq_sb = const.tile([P, DK, F], DT_IN, tag="wq")
        wk_sb = const.tile([P, DK, F], DT_IN, tag="wk")
        wv_sb = const.tile([P, DK, F], DT_IN, tag="wv")
        dve_load(wq_sb, wq_d.rearrange("(kc p) f -> p kc f", p=P),
                 [P, DK, F], DT_IN, "wq")
        dve_load(wk_sb, wk_d.rearrange("(kc p) f -> p kc f", p=P),
                 [P, DK, F], DT_IN, "wk")
        dve_load(wv_sb, wv_d.rearrange("(kc p) f -> p kc f", p=P),
                 [P, DK, F], DT_IN, "wv")
        # Wo rows split per head so contraction runs as 2 accumulating K=64
        # matmuls with partition-0-based operands (lane alignment).
        wo0_sb = const.tile([DH, D], DT_IN, tag="wo0")
        wo1_sb = const.tile([DH, D], DT_IN, tag="wo1")
        dve_load(wo0_sb, wo_d[0:DH, :], [DH, D], DT_IN, "wo0")
        dve_load(wo1_sb, wo_d[DH:2 * DH, :], [DH, D], DT_IN, "wo1")
        # mask feeds ACT (exp bias): route through an ACT copy so exp's dep
        # is ACT-program-order
        mask_raw = const.tile([P, B * 16], dt.float32, tag="mask_raw")
        nc.sync.dma_start(mask_raw, mask_d[:, :])
        mask_sb = const.tile([P, B * 16], dt.float32, tag="mask")
        nc.scalar.copy(out=mask_sb, in_=mask_raw)
        ones_sb = const.tile([P, DH], dt.float32, tag="ones")
        nc.vector.memset(ones_sb, 1.0)
        ident_g = const.tile([P, P], DT_ATTN, tag="ident_g")
        make_identity(nc, ident_g)
        ident = const.tile([P, P], DT_ATTN, tag="ident")
        nc.vector.tensor_copy(out=ident, in_=ident_g)
        VT = const.tile([P, NT], DT_ATTN, tag="VT")

        QT = const.tile([P, NT], DT_ATTN, tag="QT")
        KT = const.tile([P, NT], DT_ATTN, tag="KT")
        # V natural layout per 128-k chunk, heads split with a ones column each:
        # cols 0:64 = head0, col 64 = ones, cols 65:129 = head1, col 129 = ones
        V = const.tile([P, B * 16, 130], DT_ATTN, tag="V")
        nc.vector.memset(V[:, :, 64:65], 1.0)
        nc.vector.memset(V[:, :, 129:130], 1.0)
        # attn output (transposed, pre-Wo), one tile per head, partitions 0-63
        outT0 = const.tile([DH, NT], DT_ATTN, tag="outT0")
        outT1 = const.tile([DH, NT], DT_ATTN, tag="outT1")
        # final output staging: written once per region (no slot recycling)
        out_stage = const.tile([P, NT // P, D], DT_OUT, tag="out_stage")

        # ---- stage A: projections ----
        # kc=0's x tile flows through a DVE copy so the group-opening matmul's
        # two deps (fresh x data + psum slot recycle) merge into one DVE wait;
        # kc>0 matmuls wait only on their own x DMA lane.
        def proj(xt_d, w_sb, nts, dest):
            # nts: 1 or 2 consecutive 512-wide tiles sharing one DMA per kc
            # (256KB transfers hit much better DMA efficiency than 128KB)
            xt_r = xt_d.rearrange("(kc p) n -> p kc n", p=P)
            w = 512 * len(nts)
            n0 = nts[0] * 512
            pss = [ps_misc.tile([P, 512], dt.float32, tag="proj", bufs=B_PROJ,
                                name="ps_proj") for _ in nts]
            for kc in range(DK):
                xt = xpool.tile([P, w], DT_IN, tag=f"xt{kc}", name="xt")
                nc.sync.dma_start(xt, xt_r[:, kc, n0:n0 + w])
                if kc == 0:
                    xtc = xpool.tile([P, w], DT_IN, tag="xt0c", name="xtc")
                    nc.vector.tensor_copy(out=xtc, in_=xt)
                    xt = xtc
                for i in range(len(nts)):
                    nc.tensor.matmul(pss[i], lhsT=w_sb[:, kc, :],
                                     rhs=xt[:, i * 512:(i + 1) * 512],
                                     start=(kc == 0), stop=(kc == DK - 1))
            for i, nt in enumerate(nts):
                nc.vector.tensor_copy(out=dest[:, nt * 512:(nt + 1) * 512],
                                      in_=pss[i])

        # ---- per batch: projections then attention, so batch b+1's
        # projections (DMA/PE) overlap batch b's attention (ACT-heavy) ----
        def pairs(lst):
            return [lst[i:i + 2] for i in range(0, len(lst), 2)]

        for b in range(B):
            b_nts = [nt for nt in ntk if nt // 4 == b]
            for pr in pairs(b_nts):
                proj(xtk_d, wk_sb, pr, KT)
            for pr in pairs(b_nts):
                proj(xtv_d, wv_sb, pr, VT)
            # V natural (k on partitions) via PE transposes of VT chunks
            for kcl in range(KC[b]):
                g = b * 16 + kcl
                pst = ps_score.tile([P, P], DT_ATTN, tag="sc", name="pst")
                nc.tensor.transpose(pst, VT[:, g * 128:(g + 1) * 128], ident)
                nc.vector.tensor_copy(out=V[:, g, 0:64], in_=pst[:, 0:64])
                nc.vector.tensor_copy(out=V[:, g, 65:129], in_=pst[:, 64:128])
            for pr in pairs(list(range(4 * b, 4 * b + 4))):
                proj(xtq_d, wq_sb, pr, QT)
            for qt in range(4):
                q0 = b * S + qt * 512
                av0 = ps_av.tile([P, 512], dt.float32, tag="av", name="av0")
                av1 = ps_av.tile([P, 512], dt.float32, tag="av", name="av1")
                # zero-init on ACT (instead of start=True) so the group's
                # first matmul needs only the ACT wait; has_written stays
                # clear for PE so start=False accumulates correctly.
                nc.scalar.memzero(av0[0:65])
                nc.scalar.memzero(av1[0:65])
                for kcl in range(KC[b]):
                    g = b * 16 + kcl
                    k0 = b * S + kcl * 128
                    sc2 = ps_score.tile([P, 2, 512], dt.float32, tag="sc",
                                        name="sc2")
                    nc.tensor.matmul(sc2[:, 0, :], lhsT=KT[0:64, k0:k0 + 128],
                                     rhs=QT[0:64, q0:q0 + 512])
                    nc.tensor.matmul(sc2[:, 1, :], lhsT=KT[64:128, k0:k0 + 128],
                                     rhs=QT[64:128, q0:q0 + 512])
                    at = apool.tile([P, 2, 512], DT_ATTN, tag="at", name="at")
                    bias = mask_sb[:, g:g + 1]
                    nc.scalar.activation(at.rearrange("p a n -> p (a n)"),
                                         sc2.rearrange("p a n -> p (a n)"),
                                         mybir.ActivationFunctionType.Exp,
                                         bias=bias, scale=0.125)
                    sp = (kcl == KC[b] - 1)
                    nc.tensor.matmul(av0[0:65], lhsT=V[:, g, 0:65],
                                     rhs=at[:, 0, :], start=False, stop=sp)
                    nc.tensor.matmul(av1[0:65], lhsT=V[:, g, 65:130],
                                     rhs=at[:, 1, :], start=False, stop=sp)
                # normalize both heads: rowsum sits in row 64 of each av
                # psum. ln(rowsum) -> broadcast over 64 partitions via a K=1
                # fp32 PE matmul -> exp(-x) gives 1/rowsum on all partitions.
                for h, av, oT in ((0, av0, outT0), (1, av1, outT1)):
                    lnr = rpool.tile([65, 512], dt.float32, tag="lnr",
                                     name="lnr")
                    nc.scalar.activation(lnr[64:65, :], av[64:65, :],
                                         mybir.ActivationFunctionType.Ln)
                    bc = ps_misc.tile([P, 512], dt.float32, tag="pw",
                                      bufs=B_PW, name="bc")
                    nc.tensor.matmul(bc[0:64], lhsT=ones_sb[64:65, :],
                                     rhs=lnr[64:65, :])
                    rbc = rpool.tile([DH, 512], dt.float32, tag="rbc",
                                     name="rbc")
                    nc.scalar.activation(rbc, bc[0:64],
                                         mybir.ActivationFunctionType.Exp,
                                         scale=-1.0)
                    nc.vector.tensor_mul(out=oT[:, q0:q0 + 512],
                                         in0=av[0:64], in1=rbc)
                # Wo for this q tile: 4 s-chunks of 128, contraction split
                # into the two heads' K=64 halves (accumulated in PSUM)
                for sc in range(4):
                    gsc = (q0 + sc * 128) // 128   # global 128-row chunk
                    r0 = gsc * 128
                    for half in range(2):
                        pw = ps_misc.tile([P, 512], dt.float32, tag="pw",
                                          bufs=B_PW, name="pw")
                        w_sl = slice(half * 512, (half + 1) * 512)
                        nc.tensor.matmul(pw, lhsT=outT0[:, r0:r0 + 128],
                                         rhs=wo0_sb[:, w_sl],
                                         start=True, stop=False)
                        nc.tensor.matmul(pw, lhsT=outT1[:, r0:r0 + 128],
                                         rhs=wo1_sb[:, w_sl],
                                         start=False, stop=True)
                        nc.vector.tensor_copy(out=out_stage[:, gsc, w_sl],
                                              in_=pw)
                # one 1MB DMA out per q tile
                gs0 = q0 // 128
                nc.sync.dma_start(
                    out_d.rearrange("(g p) n -> p g n", p=P)[:, gs0:gs0 + 4, :],
                    out_stage[:, gs0:gs0 + 4, :])

    _legalize_waits(nc)
    return nc


def _legalize_waits(nc):
    """This walrus build accepts at most ONE sync-wait command per
    instruction, while Tile emits up to a dozen (e.g. the kernel-tail
    drain). Legalize by splitting: excess waits are hoisted onto
    same-engine Drain instructions inserted immediately before the
    offender — same-engine program order makes this semantically
    identical. Patched module is served via nc.to_json_bytes."""
    import json as _json

    raw = nc.to_json_bytes()
    d = _json.loads(raw)
    template = None
    for fn in d.get("functions", []):
        for blk in fn.get("blocks", []):
            for inst in blk.get("instructions", []):
                if inst.get("opcode") == "Drain":
                    template = inst
                    break
            if template:
                break
        if template:
            break
    assert template is not None, "no Drain template found"

    counter = [0]

    def carrier(engine, wait):
        counter[0] += 1
        c = _json.loads(_json.dumps(template))
        c["name"] = f"I-waitfix-{counter[0]}"
        c["engine"] = engine
        c["sync_info"] = {"on_update": [], "on_wait": [wait]}
        c["ins"] = []
        c["outs"] = []
        return c

    nfix = 0
    for fn in d.get("functions", []):
        for blk in fn.get("blocks", []):
            out = []
            for inst in blk.get("instructions", []):
                si = inst.get("sync_info")
                waits = (si or {}).get("on_wait") or []
                if len(waits) > 1:
                    for w in waits[:-1]:
                        out.append(carrier(inst["engine"], w))
                    si["on_wait"] = [waits[-1]]
                    nfix += 1
                out.append(inst)
            blk["instructions"] = out

    patched = _json.dumps(d).encode()
    nc.to_json_bytes = lambda: patched


def _fix_sync_waits(nc):
    """Walrus rejects instructions with more sync-wait commands than their
    ISA encoding holds (Matmult/Ldweights/DMACopy: 1). Tile's sem assignment
    is not transitively minimal and sometimes exceeds this. Two safe
    reductions, applied in order:

    1. Deletion by implication: drop wait W2 if a kept wait W1's producer
       instruction itself (originally) waited on the same semaphore at >= the
       required value — W1 then transitively implies W2.
    2. Relocation: move a wait onto an earlier instruction of the same
       engine (in-order, so waiting earlier is correct), positioned after
       the wait's producer (so it cannot deadlock)."""
    import bisect

    LIMITS = {"Matmult": 1, "Ldweights": 1, "DMACopy": 1}
    for f in nc.m.functions:
        for blk in f.blocks:
            insts = blk.instructions
            sem_vals: dict = {}
            sem_idx: dict = {}
            cum: dict = {}
            eng_of = []
            orig_waits = []
            for idx, inst in enumerate(insts):
                eng_of.append(str(inst.engine))
                si = inst.sync_info
                ws = []
                if si is not None:
                    ws = [(w.ant_name, w.wait_value) for w in (si.on_wait or [])]
                    for u in (si.on_update or []):
                        nm = u.ant_name
                        cum[nm] = cum.get(nm, 0) + (u.update_value or 1)
                        sem_vals.setdefault(nm, []).append(cum[nm])
                        sem_idx.setdefault(nm, []).append(idx)
                orig_waits.append(ws)

            def producer(sem, val):
                vals = sem_vals.get(sem, [])
                i = bisect.bisect_left(vals, val)
                return sem_idx[sem][i] if i < len(vals) else None

            def implied(w, kept):
                # does some kept wait's producer transitively cover w?
                seen = set()
                frontier = [(k.ant_name, k.wait_value) for k in kept]
                depth = 0
                while frontier and depth < 4:
                    nxt = []
                    for sem, val in frontier:
                        p = producer(sem, val)
                        if p is None or p in seen:
                            continue
                        seen.add(p)
                        for (s2, v2) in orig_waits[p]:
                            if s2 == w.ant_name and v2 >= w.wait_value:
                                return True
                            nxt.append((s2, v2))
                    frontier = nxt
                    depth += 1
                return False

            for idx, inst in enumerate(insts):
                if inst.opcode == "Drain":
                    continue
                si = inst.sync_info
                if si is None or not si.on_wait:
                    continue
                waits = list(si.on_wait)
                limit = LIMITS.get(inst.opcode, 2)
                if len(waits) <= limit:
                    continue
                eng = eng_of[idx]
                # keep cross-engine waits first (data deps), shed self/WAW
                self_w = [w for w in waits
                          if eng.endswith(w.ant_name.split("_")[0])]
                other_w = [w for w in waits if w not in self_w]
                ordered = other_w + self_w
                keep = ordered[:limit]
                excess = ordered[limit:]
                # try implication-deletion of excess (and also try swapping:
                # maybe a kept one is implied by an excess one)
                remaining = []
                for w in excess:
                    if implied(w, keep):
                        continue
                    swapped = False
                    for i, k in enumerate(keep):
                        trial = keep[:i] + [w] + keep[i + 1:]
                        if implied(k, trial):
                            keep = trial
                            swapped = True
                            break
                    if not swapped:
                        remaining.append(w)
                for w in remaining:
                    pidx = producer(w.ant_name, w.wait_value)
                    host = None
                    j = idx - 1
                    while j >= 0 and j > (pidx if pidx is not None else -1):
                        if eng_of[j] == eng and insts[j].opcode != "Drain":
                            hsi = insts[j].sync_info
                            hw = list(hsi.on_wait) if (
                                hsi is not None and hsi.on_wait) else []
                            hlim = LIMITS.get(insts[j].opcode, 2)
                            if hsi is not None and len(hw) < hlim and not any(
                                    x.ant_name == w.ant_name for x in hw):
                                host = (j, hsi, hw)
                                break
                        j -= 1
                    if host is None:
                        raise RuntimeError(
                            f"_fix_sync_waits: no host for {inst.name} "
                            f"wait {w.ant_name}>={w.wait_value}")
                    _, hsi, hw = host
                    hsi.on_wait = hw + [w]
                si.on_wait = keep


def _prep_host(queries, keys, values, Wq, Wk, Wv, Wo, valid_lens, cfg):
    np_in = {"bfloat16": ml_dtypes.bfloat16, "float32": np.float32,
             "float32r": np.float32, "float16": np.float16}[cfg["dt_in"]]
    L = [int(valid_lens[0]), int(valid_lens[1])]
    KC = tuple(min(16, (l + 127) // 128) for l in L)

    def t2(x):  # (B,S,D) -> (D, B*S)
        return np.ascontiguousarray(
            np.asarray(x, np.float32).reshape(NT, D).T).astype(np_in)

    xtq, xtk, xtv = t2(queries), t2(keys), t2(values)
    maskt = np.full((P, B * 16), NEG, np.float32)
    for b in range(B):
        for c in range(16):
            ks = c * 128 + np.arange(P)
            maskt[:, b * 16 + c] = np.where(ks < L[b], 0.0, NEG)

    Wq = np.asarray(Wq, np.float32)
    Wk = np.asarray(Wk, np.float32)
    Wv = np.asarray(Wv, np.float32)
    Wo = np.asarray(Wo, np.float32)
    in_maps = []
    for c in range(N_CORES):
        cs = slice(c * F, (c + 1) * F)
        in_maps.append({
            "xtq": xtq, "xtk": xtk, "xtv": xtv,
            "wq": np.ascontiguousarray(Wq[:, cs]).astype(np_in),
            "wk": np.ascontiguousarray(Wk[:, cs]).astype(np_in),
            "wv": np.ascontiguousarray(Wv[:, cs]).astype(np_in),
            "wo": np.ascontiguousarray(Wo[cs, :]).astype(np_in),
            "maskt": maskt,
        })
    return KC, in_maps


DEFAULT_CFG = {"dt_in": "float16", "dt_attn": "float16", "dt_out": "float16"}

LAST_RESULTS = None


def kernel(queries, keys, values, Wq, Wk, Wv, Wo, valid_lens):
    global LAST_RESULTS
    from concourse.bass_utils import run_bass_kernel_spmd

    cfg = dict(DEFAULT_CFG)
    if os.environ.get("MHA_CFG"):
        for kv in os.environ["MHA_CFG"].split(","):
            k, v = kv.split("=")
            cfg[k] = v

    KC, in_maps = _prep_host(queries, keys, values, Wq, Wk, Wv, Wo,
                             valid_lens, cfg)
    key = (KC, tuple(sorted(cfg.items())))
    if key not in _CACHE:
        _CACHE[key] = _build_program(KC, cfg)
    nc = _CACHE[key]

    trace = bool(os.environ.get("MHA_TRACE"))
    res = run_bass_kernel_spmd(nc, in_maps, core_ids=list(range(N_CORES)),
                               trace=trace)
    LAST_RESULTS = res
    acc = np.zeros((NT, D), np.float32)
    for r in res.results:
        acc += np.asarray(r["out_part"], np.float32)
    return acc.reshape(B, S, D)



# revision 21
# speedup vs baseline: 1.2938x; 1.2938x over previous
"""Multi-head attention (16 heads, D=1024, B=2, S=2048) on 8 TRN2 NeuronCores.

Sharding: tensor-parallel over heads. Each core owns 2 heads (128 features):
W_q/k/v column-sliced, W_o row-sliced; partial outputs summed on host.

Device dataflow (per core):
  QT[f,s], KT[f,s] = W^T x^T   (feat-major projections, contraction on parts)
  V[k,f]           = x W       (token-major projection, k on partitions)
  scores^T[k,q] = KT_h^T . QT_h  per 128-k chunk, both heads -> one psum pair
  attn^T = exp(scores*0.125 + mask_bias[k])  (ACT, psum->SBUF fp16)
  av[q, 65] += attn_chunk_h^T . [V_h | 1]    (N=65 matmuls, psum accum over k;
        col 64 = softmax denominator, per-partition in q!)
  recip = 1/rowsum (DVE), attn_out[q,f] = av * recip  (tensor_scalar per-part)
  outT[f,q] via PE transpose;  out_part[s,:] = outT^T . Wo  -> fp16, host sums

Key-padding mask: k-chunks beyond valid_len are skipped (program specialized
on valid_lens); boundary chunk masked via -1e6 exp bias (underflows to 0).

Build order = schedule: engines run in program order, so proj(b+1) groups and
the qt Wo-tails are interleaved into the attn chunk loop as PE filler to keep
ACT (the exp bottleneck) saturated.

cfg proj8=1: x and W_q/k/v in fp8e4 with DoubleRow matmuls (2x PE rate,
half the x DMA bytes). Layout is a pure AP rearrange: contraction index
d = c*256 + two*128 + p for both weights and x, so the sum is unchanged.
"""

import math
import os

import ml_dtypes
import numpy as np

B = 2
S = 2048
D = 1024
NT = B * S          # 4096 rows, b-major
F = 128             # features per core (2 heads x 64)
DH = 64
P = 128
DK = D // P         # 8 contraction chunks for projections
N_CORES = 8
NEG = -1e6

_CACHE: dict = {}


def _build_program(KC: tuple[int, int], cfg: dict):
    import concourse.bass as bass
    import concourse.tile as tile
    from concourse import mybir
    from concourse.masks import make_identity

    dt = mybir.dt
    DT_IN = getattr(dt, cfg["dt_in"])        # W_o / non-fp8 operands
    DT_ATTN = getattr(dt, cfg["dt_attn"])    # attn / V / QT / KT storage
    DT_OUT = getattr(dt, cfg["dt_out"])      # partial output in HBM
    PROJ8 = str(cfg.get("proj8", "0")) == "1"
    DT_X = dt.float8e4 if PROJ8 else DT_IN
    PM = mybir.MatmulPerfMode.DoubleRow if PROJ8 else None
    Exp = mybir.ActivationFunctionType.Exp

    nc = bass.Bass("TRN2")

    xtq_d = nc.dram_tensor("xtq", [D, NT], DT_X, kind="ExternalInput")
    xtk_d = nc.dram_tensor("xtk", [D, NT], DT_X, kind="ExternalInput")
    xtv_d = nc.dram_tensor("xtv", [D, NT], DT_X, kind="ExternalInput")
    wq_d = nc.dram_tensor("wq", [D, F], DT_X, kind="ExternalInput")
    wk_d = nc.dram_tensor("wk", [D, F], DT_X, kind="ExternalInput")
    wv_d = nc.dram_tensor("wv", [D, F], DT_X, kind="ExternalInput")
    wo_d = nc.dram_tensor("wo", [F, D], DT_IN, kind="ExternalInput")
    mask_d = nc.dram_tensor("maskt", [P, B * 16], dt.float32, kind="ExternalInput")
    out_d = nc.dram_tensor("out_part", [NT, D], DT_OUT, kind="ExternalOutput")

    KW = [KC[0] * 128, KC[1] * 128]   # K/V token count per batch

    from contextlib import ExitStack

    with tile.TileContext(nc) as tc, ExitStack() as ctx:
        const = ctx.enter_context(tc.tile_pool(name="const", bufs=1))
        xpool = ctx.enter_context(tc.tile_pool(name="xpool", bufs=12))
        apool = ctx.enter_context(tc.tile_pool(name="apool", bufs=4))
        aopool = ctx.enter_context(tc.tile_pool(name="aopool", bufs=5))
        otpool = ctx.enter_context(tc.tile_pool(name="otpool", bufs=2))
        ospool = ctx.enter_context(tc.tile_pool(name="ospool", bufs=2))
        rpool = ctx.enter_context(tc.tile_pool(name="rpool", bufs=4))
        ps_sc = ctx.enter_context(
            tc.tile_pool(name="ps_sc", bufs=2, space="PSUM"))
        ps_av = ctx.enter_context(
            tc.tile_pool(name="ps_av", bufs=1, space="PSUM"))
        ps_wo = ctx.enter_context(
            tc.tile_pool(name="ps_wo", bufs=2, space="PSUM"))

        # ---- constants ----
        # Matmult instructions tolerate only ONE sync-wait; weight/identity
        # loads bounce DRAM -> raw tile -> DVE copy so matmul deps merge.
        def dve_load(dst, src_ap, raw_shape, raw_dtype, nm):
            raw = const.tile(list(raw_shape), raw_dtype, tag=f"{nm}_raw",
                             name=f"{nm}_raw")
            nc.sync.dma_start(raw, src_ap)
            nc.vector.tensor_copy(out=dst, in_=raw)

        if PROJ8:
            wsh = [P, DK // 2, 2, F]
            def wre(wd):
                return wd.rearrange("(c two p) f -> p c two f", p=P, two=2)
        else:
            wsh = [P, DK, F]
            def wre(wd):
                return wd.rearrange("(kc p) f -> p kc f", p=P)
        wq_sb = const.tile(wsh, DT_X, tag="wq")
        wk_sb = const.tile(wsh, DT_X, tag="wk")
        wv_sb = const.tile(wsh, DT_X, tag="wv")
        dve_load(wk_sb, wre(wk_d), wsh, DT_X, "wk")
        dve_load(wv_sb, wre(wv_d), wsh, DT_X, "wv")
        dve_load(wq_sb, wre(wq_d), wsh, DT_X, "wq")
        # mask feeds ACT (exp bias): route through an ACT copy so exp's dep
        # is ACT-program-order
        mask_raw = const.tile([P, B * 16], dt.float32, tag="mask_raw")
        nc.sync.dma_start(mask_raw, mask_d[:, :])
        mask_sb = const.tile([P, B * 16], dt.float32, tag="mask")
        nc.scalar.copy(out=mask_sb, in_=mask_raw)
        # wo / identity are needed only by the first qt tail (~30us in);
        # emitted after the startup x DMAs so they don't delay them
        wo_sb = const.tile([F, D], DT_IN, tag="wo")
        ident = const.tile([P, P], DT_ATTN, tag="ident")

        def late_consts():
            dve_load(wo_sb, wo_d[:, :], [F, D], DT_IN, "wo")
            ident_g = const.tile([P, P], DT_ATTN, tag="ident_g")
            make_identity(nc, ident_g)
            nc.vector.tensor_copy(out=ident, in_=ident_g)

        # PE warmup: a few junk matmuls anchor the p-state ramp so the
        # first projections run at full clock (scratch psum, never read)
        warm = const.tile([P, 512], DT_ATTN, tag="warm")
        nc.vector.memset(warm, 1.0)
        for _ in range(6):
            wps = ps_sc.tile([P, 512], dt.float32, tag="sc", name="wps")
            nc.tensor.matmul(wps, lhsT=warm[:, 0:128], rhs=warm)

        QT = const.tile([P, NT], DT_ATTN, tag="QT")
        KT = const.tile([P, NT], DT_ATTN, tag="KT")
        # V natural layout (k on partitions) per 128-k chunk:
        # cols 0:64 = head0, col 64 = ones, cols 65:129 = head1, col 129 = ones
        V = const.tile([P, B * 16, 130], DT_ATTN, tag="V")
        nc.vector.memset(V[:, :, 64:65], 1.0)
        nc.vector.memset(V[:, :, 129:130], 1.0)

        # ---- x staging: one tile per 512-token chunk, created lazily in
        # its DMA closure so ring order == issue order; projections look the
        # quarter tile up at emission time (always after its DMA) ----
        def x_chunks(xd, b, w, nm):
            if PROJ8:
                xr = xd.rearrange("(c two p) n -> p c two n", p=P, two=2)
            else:
                xr = xd.rearrange("(kc p) n -> p kc n", p=P)
            n0 = b * S
            tiles = {}
            dmas = []
            for j, t0 in enumerate(range(0, w, 512)):
                tw = min(512, w - t0)

                def mk(j=j, t0=t0, tw=tw):
                    if PROJ8:
                        st = xpool.tile([P, DK // 2, 2, tw], DT_X, tag="xst",
                                        name=f"{nm}_{j}")
                        nc.sync.dma_start(st,
                                          xr[:, :, :, n0 + t0:n0 + t0 + tw])
                    else:
                        st = xpool.tile([P, DK, tw], DT_X, tag="xst",
                                        name=f"{nm}_{j}")
                        nc.sync.dma_start(st,
                                          xr[:, :, n0 + t0:n0 + t0 + tw])
                    tiles[j] = st
                dmas.append(mk)
            return tiles.get, dmas

        def proj_feat_group(x_get, q, tw, w_sb, dst, dst0, off=0):
            # one ≤512-wide output tile of a feat-major projection from
            # quarter tile q (dst cols dst0+512q+off ...)
            x_sb = x_get(q)
            c0 = dst0 + q * 512 + off
            ps = ps_sc.tile([P, 512], dt.float32, tag="sc", name="ps_pr")
            if PROJ8:
                for c in range(DK // 2):
                    nc.tensor.matmul(ps[:, :tw], lhsT=w_sb[:, c, :, :],
                                     rhs=x_sb[:, c, :, off:off + tw],
                                     start=(c == 0), stop=(c == DK // 2 - 1),
                                     perf_mode=PM)
            else:
                for kc in range(DK):
                    nc.tensor.matmul(ps[:, :tw], lhsT=w_sb[:, kc, :],
                                     rhs=x_sb[:, kc, off:off + tw],
                                     start=(kc == 0), stop=(kc == DK - 1))
            nc.vector.tensor_copy(out=dst[:, c0:c0 + tw], in_=ps[:, :tw])

        def proj_tok_group(x_get, w_sb, b, kcl):
            # one 128-token chunk of the token-major V projection
            g = b * 16 + kcl
            x_sb = x_get(kcl // 4)
            t0 = (kcl % 4) * 128
            ps = ps_sc.tile([P, F], dt.float32, tag="sc", name="ps_v")
            if PROJ8:
                for c in range(DK // 2):
                    nc.tensor.matmul(ps, lhsT=x_sb[:, c, :, t0:t0 + 128],
                                     rhs=w_sb[:, c, :, :],
                                     start=(c == 0), stop=(c == DK // 2 - 1),
                                     perf_mode=PM)
            else:
                for kc in range(DK):
                    nc.tensor.matmul(ps, lhsT=x_sb[:, kc, t0:t0 + 128],
                                     rhs=w_sb[:, kc, :],
                                     start=(kc == 0), stop=(kc == DK - 1))
            nc.vector.tensor_copy(out=V[:, g, 0:64], in_=ps[:, 0:64])
            nc.vector.tensor_copy(out=V[:, g, 65:129], in_=ps[:, 64:128])

        def tiles_of(w):
            return [(t0, min(512, w - t0)) for t0 in range(0, w, 512)]

        # filler queues: closures emitting one PE work group each, drained
        # into the attn chunk loop's exp-wait gaps. Tails drain only from
        # kcl>=2 so their transposes don't block PE on the qt-boundary
        # normalize chain.
        tail_q: list = []
        bulk_q: list = []

        def drain_one(kcl=2):
            if kcl >= 2 and tail_q:
                tail_q.pop(0)()
            elif bulk_q:
                bulk_q.pop(0)()
            elif kcl >= 2 and tail_q:
                tail_q.pop(0)()

        def qt_tail_items(b, qt, aos, split_dma=False, use_act=False):
            # the Wo tail split into 5 small filler items so it drains into
            # chunk-loop slack instead of stalling ACT at the qt boundary
            q0 = b * S + qt * 512
            cell = {}

            def item0():
                oT = otpool.tile([P, 512], DT_ATTN, tag="oT", name="oT")
                for qs in range(4):
                    tr = ps_wo.tile([P, P], DT_ATTN, tag="pw", name="tr")
                    nc.tensor.transpose(tr, aos[qs], ident)
                    nc.vector.tensor_copy(out=oT[:, qs * 128:(qs + 1) * 128],
                                          in_=tr)
                cell["oT"] = oT
                cell["ost"] = ospool.tile([P, 4, D], DT_OUT, tag="ost",
                                          name="ost")

            def mk_wo(sc4):
                def item():
                    oT, ost = cell["oT"], cell["ost"]
                    for half in range(2):
                        pw = ps_wo.tile([P, 512], dt.float32, tag="pw",
                                        name="pw")
                        nc.tensor.matmul(
                            pw, lhsT=oT[:, sc4 * 128:(sc4 + 1) * 128],
                            rhs=wo_sb[:, half * 512:(half + 1) * 512])
                        dst = ost[:, sc4, half * 512:(half + 1) * 512]
                        if use_act and half == 0:
                            nc.scalar.copy(out=dst, in_=pw)
                        else:
                            nc.vector.tensor_copy(out=dst, in_=pw)
                    gs0 = q0 // 128
                    orr = out_d.rearrange("(g p) n -> p g n", p=P)
                    if split_dma and sc4 == 1:
                        nc.sync.dma_start(orr[:, gs0:gs0 + 2, :],
                                          cell["ost"][:, 0:2, :])
                    if sc4 == 3:
                        if split_dma:
                            nc.sync.dma_start(orr[:, gs0 + 2:gs0 + 4, :],
                                              cell["ost"][:, 2:4, :])
                        else:
                            nc.sync.dma_start(orr[:, gs0:gs0 + 4, :],
                                              cell["ost"])
                return item
            return [item0, mk_wo(0), mk_wo(1), mk_wo(2), mk_wo(3)]

        # ---- schedule ----
        # b0: qt0's chunk loop doubles as the startup ramp — K/V quarter
        # DMAs + projections are embedded so exp starts after ~3 chunk DMAs.
        # b1: x DMAs and proj groups become filler drained through b0's attn.
        # qt Wo-tails are deferred one qt (front of the filler queue).
        def attn_qt(b, qt, embed=None, late=None, last=False):
            q0 = b * S + qt * 512
            # zero-init on DVE instead of start=True: the pending-zero
            # region is a full 2KB bank, so per-qslice start=True matmuls
            # would wipe their neighbours' chunk-0 accumulations
            av0 = ps_av.tile([P, 4, 65], dt.float32, tag="av0", name="av0")
            av1 = ps_av.tile([P, 4, 65], dt.float32, tag="av1", name="av1")
            nc.vector.memset(av0, 0.0)
            nc.vector.memset(av1, 0.0)
            pend = None
            for kcl in range(KC[b]):
                if embed is not None:
                    embed(kcl)
                if late is not None and kcl == max(0, KC[b] - 2):
                    late()
                g = b * 16 + kcl
                k0 = b * S + kcl * 128
                sc2 = ps_sc.tile([P, 2, 512], dt.float32, tag="sc",
                                 name="sc2")
                nc.tensor.matmul(sc2[:, 0, :], lhsT=KT[0:64, k0:k0 + 128],
                                 rhs=QT[0:64, q0:q0 + 512])
                nc.tensor.matmul(sc2[:, 1, :], lhsT=KT[64:128, k0:k0 + 128],
                                 rhs=QT[64:128, q0:q0 + 512])
                at = apool.tile([P, 2, 512], DT_ATTN, tag="at", name="at")
                nc.scalar.activation(at.rearrange("p a n -> p (a n)"),
                                     sc2.rearrange("p a n -> p (a n)"),
                                     Exp, bias=mask_sb[:, g:g + 1],
                                     scale=0.125)
                # AV of the previous chunk runs after this chunk's scores so
                # PE never blocks on the current exp (1-deep pipeline)
                if pend is not None:
                    pend()
                if embed is None and kcl >= 1:
                    drain_one(kcl)

                def mk_av(at=at, g=g, sp=(kcl == KC[b] - 1)):
                    for qs in range(4):
                        nc.tensor.matmul(
                            av0[:, qs, :],
                            lhsT=at[:, 0, qs * 128:(qs + 1) * 128],
                            rhs=V[:, g, 0:65], start=False, stop=sp)
                        nc.tensor.matmul(
                            av1[:, qs, :],
                            lhsT=at[:, 1, qs * 128:(qs + 1) * 128],
                            rhs=V[:, g, 65:130], start=False, stop=sp)
                pend = mk_av
            pend()
            # softmax denominators are per-partition (col 64): normalize
            rc0 = rpool.tile([P, 4, 1], dt.float32, tag="rc", name="rc0")
            rc1 = rpool.tile([P, 4, 1], dt.float32, tag="rc", name="rc1")
            nc.vector.reciprocal(rc0, av0[:, :, 64:65])
            nc.vector.reciprocal(rc1, av1[:, :, 64:65])
            aos = []
            for qs in range(4):
                ao = aopool.tile([P, P], DT_ATTN, tag="ao", name="ao")
                nc.vector.tensor_scalar_mul(ao[:, 0:64], av0[:, qs, 0:64],
                                            rc0[:, qs, :])
                nc.vector.tensor_scalar_mul(ao[:, 64:128], av1[:, qs, 0:64],
                                            rc1[:, qs, :])
                aos.append(ao)
            if last:
                for it in qt_tail_items(b, qt, aos, split_dma=True,
                                        use_act=True):
                    it()
            else:
                # defer the Wo tail into the next qt's chunk loop
                tail_q.extend(qt_tail_items(b, qt, aos))

        # --- batch 0 startup ---
        gV0, pV0 = x_chunks(xtv_d, 0, KW[0], "xv0")
        gK0, pK0 = x_chunks(xtk_d, 0, KW[0], "xk0")
        gQ0, pQ0 = x_chunks(xtq_d, 0, S, "xq0")
        nqKV = len(pV0)
        # prologue: K quarter 0 first (first scores need only K chunk 0 and
        # Q tile 0), V needed one iteration later
        pK0[0]()
        pQ0[0]()
        pV0[0]()
        if nqKV > 1:
            pK0[1]()
            pV0[1]()
        late_consts()
        kt_tiles = tiles_of(KW[0])

        def embed0(kcl):
            if kcl % 4 == 0:
                q = kcl // 4
                if q + 2 < nqKV:
                    pK0[q + 2]()
                    pV0[q + 2]()
                if kcl == 8 or (KC[0] <= 8 and kcl == 0):
                    for t in range(1, 4):
                        pQ0[t]()
                if q > 0 and q < len(kt_tiles):
                    proj_feat_group(gK0, q, kt_tiles[q][1], wk_sb, KT, 0)
            if kcl == 0:
                # minimal path to the first exp: K chunk 0 only, then Q t0
                proj_feat_group(gK0, 0, min(128, KW[0]), wk_sb, KT, 0)
                proj_feat_group(gQ0, 0, 512, wq_sb, QT, 0)
                proj_tok_group(gV0, wv_sb, 0, 0)
                if KC[0] > 1:
                    proj_tok_group(gV0, wv_sb, 0, 1)
                if KW[0] > 128:
                    proj_feat_group(gK0, 0, min(KW[0], 512) - 128, wk_sb,
                                    KT, 0, off=128)
            elif kcl + 1 < KC[0]:
                proj_tok_group(gV0, wv_sb, 0, kcl + 1)
            if KC[0] <= 8 and kcl == KC[0] - 1:
                for t in range(1, 4):
                    pQ0[t]()

        attn_qt(0, 0, embed=embed0,
                late=lambda: proj_feat_group(gQ0, 1, 512, wq_sb, QT, 0))

        # --- batch 1 prefetch as filler (drained through b0 qt1-3) ---
        if B > 1:
            gV1, pV1 = x_chunks(xtv_d, 1, KW[1], "xv1")
            gK1, pK1 = x_chunks(xtk_d, 1, KW[1], "xk1")
            gQ1, pQ1 = x_chunks(xtq_d, 1, S, "xq1")
            items = []
            kt1 = tiles_of(KW[1])
            nq1 = len(pV1)

            def v_pair(c0):
                def it():
                    for c in range(c0, min(c0 + 2, KC[1])):
                        proj_tok_group(gV1, wv_sb, 1, c)
                return it
            for j in range(nq1):
                items.append(pK1[j])
                items.append(pV1[j])
                if j >= 1:
                    jj = j - 1
                    items.append((lambda q, d: lambda: proj_feat_group(
                        gK1, q, d, wk_sb, KT, S))(jj, kt1[jj][1]))
                    for c0 in range(4 * jj, min(4 * jj + 4, KC[1]), 2):
                        items.append(v_pair(c0))
            items.append((lambda q, d: lambda: proj_feat_group(
                gK1, q, d, wk_sb, KT, S))(nq1 - 1, kt1[-1][1]))
            for c0 in range(max(0, 4 * (nq1 - 1)), KC[1], 2):
                items.append(v_pair(c0))
            for t in range(4):
                items.append(pQ1[t])
                if t >= 1:
                    items.append((lambda q: lambda: proj_feat_group(
                        gQ1, q, 512, wq_sb, QT, S))(t - 1))
            items.append((lambda: lambda: proj_feat_group(
                gQ1, 3, 512, wq_sb, QT, S))())
            bulk_q.extend(items)

        attn_qt(0, 1,
                late=lambda: proj_feat_group(gQ0, 2, 512, wq_sb, QT, 0))
        attn_qt(0, 2,
                late=lambda: proj_feat_group(gQ0, 3, 512, wq_sb, QT, 0))
        attn_qt(0, 3)
        # batch-1 attention reads KT/V/QT(b1): flush any un-drained
        # projection work before the first read is emitted
        while bulk_q:
            drain_one(0)
        for qt in range(4):
            attn_qt(1, qt, last=(qt == 3))
        while tail_q or bulk_q:
            drain_one(2)

    _legalize_waits(nc)
    return nc


def _legalize_waits(nc):
    """This walrus build accepts at most ONE sync-wait command per
    instruction, while Tile emits up to a dozen (e.g. the kernel-tail
    drain). Legalize by splitting: excess waits are hoisted onto
    same-engine Drain instructions inserted immediately before the
    offender — same-engine program order makes this semantically
    identical. Patched module is served via nc.to_json_bytes."""
    import json as _json

    raw = nc.to_json_bytes()
    d = _json.loads(raw)
    template = None
    for fn in d.get("functions", []):
        for blk in fn.get("blocks", []):
            for inst in blk.get("instructions", []):
                if inst.get("opcode") == "Drain":
                    template = inst
                    break
            if template:
                break
        if template:
            break
    assert template is not None, "no Drain template found"

    counter = [0]

    def carrier(engine, wait):
        counter[0] += 1
        c = _json.loads(_json.dumps(template))
        c["name"] = f"I-waitfix-{counter[0]}"
        c["engine"] = engine
        c["sync_info"] = {"on_update": [], "on_wait": [wait]}
        c["ins"] = []
        c["outs"] = []
        return c

    nfix = 0
    for fn in d.get("functions", []):
        for blk in fn.get("blocks", []):
            out = []
            for inst in blk.get("instructions", []):
                si = inst.get("sync_info")
                waits = (si or {}).get("on_wait") or []
                if len(waits) > 1:
                    for w in waits[:-1]:
                        out.append(carrier(inst["engine"], w))
                    si["on_wait"] = [waits[-1]]
                    nfix += 1
                out.append(inst)
            blk["instructions"] = out

    patched = _json.dumps(d).encode()
    nc.to_json_bytes = lambda: patched


def _prep_host(queries, keys, values, Wq, Wk, Wv, Wo, valid_lens, cfg):
    np_map = {"bfloat16": ml_dtypes.bfloat16, "float32": np.float32,
              "float16": np.float16}
    PROJ8 = str(cfg.get("proj8", "0")) == "1"
    np_in = np_map[cfg["dt_in"]]
    np_x = ml_dtypes.float8_e4m3 if PROJ8 else np_in
    L = [int(valid_lens[0]), int(valid_lens[1])]
    KC = tuple(min(16, (l + 127) // 128) for l in L)

    def t2(x):  # (B,S,D) -> (D, B*S)
        return np.ascontiguousarray(
            np.asarray(x, np.float32).reshape(NT, D).T).astype(np_x)

    xtq, xtk, xtv = t2(queries), t2(keys), t2(values)
    maskt = np.full((P, B * 16), NEG, np.float32)
    for b in range(B):
        for c in range(16):
            ks = c * 128 + np.arange(P)
            maskt[:, b * 16 + c] = np.where(ks < L[b], 0.0, NEG)

    Wq = np.asarray(Wq, np.float32)
    Wk = np.asarray(Wk, np.float32)
    Wv = np.asarray(Wv, np.float32)
    Wo = np.asarray(Wo, np.float32)
    in_maps = []
    for c in range(N_CORES):
        cs = slice(c * F, (c + 1) * F)
        in_maps.append({
            "xtq": xtq, "xtk": xtk, "xtv": xtv,
            "wq": np.ascontiguousarray(Wq[:, cs]).astype(np_x),
            "wk": np.ascontiguousarray(Wk[:, cs]).astype(np_x),
            "wv": np.ascontiguousarray(Wv[:, cs]).astype(np_x),
            "wo": np.ascontiguousarray(Wo[cs, :]).astype(np_in),
            "maskt": maskt,
        })
    return KC, in_maps


DEFAULT_CFG = {"dt_in": "float16", "dt_attn": "float16", "dt_out": "float16",
               "proj8": "0"}

LAST_RESULTS = None


def kernel(queries, keys, values, Wq, Wk, Wv, Wo, valid_lens):
    global LAST_RESULTS
    from concourse.bass_utils import run_bass_kernel_spmd

    cfg = dict(DEFAULT_CFG)
    if os.environ.get("MHA_CFG"):
        for kv in os.environ["MHA_CFG"].split(","):
            k, v = kv.split("=")
            cfg[k] = v

    KC, in_maps = _prep_host(queries, keys, values, Wq, Wk, Wv, Wo,
                             valid_lens, cfg)
    key = (KC, tuple(sorted(cfg.items())))
    if key not in _CACHE:
        _CACHE[key] = _build_program(KC, cfg)
    nc = _CACHE[key]

    trace = bool(os.environ.get("MHA_TRACE"))
    res = run_bass_kernel_spmd(nc, in_maps, core_ids=list(range(N_CORES)),
                               trace=trace)
    LAST_RESULTS = res
    acc = np.zeros((NT, D), np.float32)
    for r in res.results:
        acc += np.asarray(r["out_part"], np.float32)
    return acc.reshape(B, S, D)


# revision 30
# speedup vs baseline: 1.3468x; 1.0410x over previous
"""Multi-head attention (16 heads, D=1024, B=2, S=2048) on 8 TRN2 NeuronCores.

Sharding: tensor-parallel over heads. Each core owns 2 heads (128 features):
W_q/k/v column-sliced, W_o row-sliced; partial outputs summed on host.

Device dataflow (per core):
  QT[f,s], KT[f,s] = W^T x^T   (feat-major projections, contraction on parts)
  V[k,f]           = x W       (token-major projection, k on partitions)
  scores^T[k,q] = KT_h^T . QT_h  per 128-k chunk, both heads -> one psum pair
  attn^T = exp(scores*0.125 + mask_bias[k])  (ACT, psum->SBUF fp16)
  av[q, 65] += attn_chunk_h^T . [V_h | 1]    (N=65 matmuls, psum accum over k;
        col 64 = softmax denominator, per-partition in q!)
  recip = 1/rowsum (DVE), attn_out[q,f] = av * recip  (tensor_scalar per-part)
  outT[f,q] via PE transpose;  out_part[s,:] = outT^T . Wo  -> fp16, host sums

Key-padding mask: k-chunks beyond valid_len are skipped (program specialized
on valid_lens); boundary chunk masked via -1e6 exp bias (underflows to 0).

Build order = schedule: engines run in program order, so proj(b+1) groups and
the qt Wo-tails are interleaved into the attn chunk loop as PE filler to keep
ACT (the exp bottleneck) saturated.

cfg proj8=1: x and W_q/k/v in fp8e4 with DoubleRow matmuls (2x PE rate,
half the x DMA bytes). Layout is a pure AP rearrange: contraction index
d = c*256 + two*128 + p for both weights and x, so the sum is unchanged.
"""

import math
import os

import ml_dtypes
import numpy as np

B = 2
S = 2048
D = 1024
NT = B * S          # 4096 rows, b-major
F = 128             # features per core (2 heads x 64)
DH = 64
P = 128
DK = D // P         # 8 contraction chunks for projections
N_CORES = 8
NEG = -1e6

_CACHE: dict = {}


def _build_program(KC: tuple[int, int], cfg: dict):
    import concourse.bass as bass
    import concourse.tile as tile
    from concourse import mybir
    from concourse.masks import make_identity

    dt = mybir.dt
    DT_IN = getattr(dt, cfg["dt_in"])        # W_o / non-fp8 operands
    DT_ATTN = getattr(dt, cfg["dt_attn"])    # attn / V / QT / KT storage
    DT_OUT = getattr(dt, cfg["dt_out"])      # partial output in HBM
    p8 = str(cfg.get("proj8", "0"))
    F8 = {"q": p8 in ("1", "qk", "q"), "k": p8 in ("1", "qk", "q"),
          "v": p8 == "1"}
    def dtx(t):
        return dt.float8e4 if F8[t] else DT_IN
    PM = mybir.MatmulPerfMode.DoubleRow
    Exp = mybir.ActivationFunctionType.Exp

    nc = bass.Bass("TRN2")

    xtq_d = nc.dram_tensor("xtq", [D, NT], dtx("q"), kind="ExternalInput")
    xtk_d = nc.dram_tensor("xtk", [D, NT], dtx("k"), kind="ExternalInput")
    xtv_d = nc.dram_tensor("xtv", [D, NT], dtx("v"), kind="ExternalInput")
    wq_d = nc.dram_tensor("wq", [D, F], dtx("q"), kind="ExternalInput")
    wk_d = nc.dram_tensor("wk", [D, F], dtx("k"), kind="ExternalInput")
    wv_d = nc.dram_tensor("wv", [D, F], dtx("v"), kind="ExternalInput")
    wo_d = nc.dram_tensor("wo", [F, D], DT_IN, kind="ExternalInput")
    mask_d = nc.dram_tensor("maskt", [P, B * 16], dt.float32, kind="ExternalInput")
    out_d = nc.dram_tensor("out_part", [NT, D], DT_OUT, kind="ExternalOutput")

    KW = [KC[0] * 128, KC[1] * 128]   # K/V token count per batch

    from contextlib import ExitStack

    with tile.TileContext(nc) as tc, ExitStack() as ctx:
        const = ctx.enter_context(tc.tile_pool(name="const", bufs=1))
        xpool = ctx.enter_context(tc.tile_pool(name="xpool", bufs=12))
        apool = ctx.enter_context(tc.tile_pool(name="apool", bufs=4))
        aopool = ctx.enter_context(tc.tile_pool(name="aopool", bufs=5))
        otpool = ctx.enter_context(tc.tile_pool(name="otpool", bufs=2))
        ospool = ctx.enter_context(tc.tile_pool(name="ospool", bufs=2))
        rpool = ctx.enter_context(tc.tile_pool(name="rpool", bufs=4))
        ps_sc = ctx.enter_context(
            tc.tile_pool(name="ps_sc", bufs=2, space="PSUM"))
        ps_av = ctx.enter_context(
            tc.tile_pool(name="ps_av", bufs=1, space="PSUM"))
        ps_wo = ctx.enter_context(
            tc.tile_pool(name="ps_wo", bufs=2, space="PSUM"))

        # ---- constants ----
        # Matmult instructions tolerate only ONE sync-wait; weight/identity
        # loads bounce DRAM -> raw tile -> DVE copy so matmul deps merge.
        def dve_load(dst, src_ap, raw_shape, raw_dtype, nm):
            raw = const.tile(list(raw_shape), raw_dtype, tag=f"{nm}_raw",
                             name=f"{nm}_raw")
            nc.sync.dma_start(raw, src_ap)
            nc.vector.tensor_copy(out=dst, in_=raw)

        def wsh(t):
            return [P, DK // 2, 2, F] if F8[t] else [P, DK, F]

        def wre(wd, t):
            if F8[t]:
                return wd.rearrange("(c two p) f -> p c two f", p=P, two=2)
            return wd.rearrange("(kc p) f -> p kc f", p=P)
        wq_sb = const.tile(wsh("q"), dtx("q"), tag="wq")
        wk_sb = const.tile(wsh("k"), dtx("k"), tag="wk")
        wv_sb = const.tile(wsh("v"), dtx("v"), tag="wv")
        mask_sb = const.tile([P, B * 16], dt.float32, tag="mask")
        # wo / identity are needed only by the first qt tail (~30us in);
        # emitted after the startup x DMAs so they don't delay them
        wo_sb = const.tile([F, D], DT_IN, tag="wo")
        ident = const.tile([P, P], DT_ATTN, tag="ident")

        def late_consts():
            dve_load(wo_sb, wo_d[:, :], [F, D], DT_IN, "wo")
            ident_g = const.tile([P, P], DT_ATTN, tag="ident_g")
            make_identity(nc, ident_g)
            nc.vector.tensor_copy(out=ident, in_=ident_g)

        # PE warmup: a few junk matmuls anchor the p-state ramp so the
        # first projections run at full clock (scratch psum, never read)
        warm = const.tile([P, 512], DT_ATTN, tag="warm")
        nc.vector.memset(warm, 1.0)
        for _ in range(6):
            wps = ps_sc.tile([P, 512], dt.float32, tag="sc", name="wps")
            nc.tensor.matmul(wps, lhsT=warm[:, 0:128], rhs=warm)

        QT = const.tile([P, NT], DT_ATTN, tag="QT")
        KT = const.tile([P, NT], DT_ATTN, tag="KT")
        # V natural layout (k on partitions) per 128-k chunk:
        # cols 0:64 = head0, col 64 = ones, cols 65:129 = head1, col 129 = ones
        V = const.tile([P, B * 16, 130], DT_ATTN, tag="V")
        nc.vector.memset(V[:, :, 64:65], 1.0)
        nc.vector.memset(V[:, :, 129:130], 1.0)

        # ---- x staging: one tile per 512-token chunk, created lazily in
        # its DMA closure so ring order == issue order; projections look the
        # quarter tile up at emission time (always after its DMA) ----
        def x_chunks(xd, b, w, nm, t):
            f8 = F8[t]
            if f8:
                xr = xd.rearrange("(c two p) n -> p c two n", p=P, two=2)
            else:
                xr = xd.rearrange("(kc p) n -> p kc n", p=P)
            n0 = b * S
            tiles = {}
            dmas = []
            for j, t0 in enumerate(range(0, w, 512)):
                tw = min(512, w - t0)

                def mk(j=j, t0=t0, tw=tw):
                    if f8:
                        st = xpool.tile([P, DK // 2, 2, tw], dtx(t),
                                        tag="xst", name=f"{nm}_{j}")
                        nc.sync.dma_start(st,
                                          xr[:, :, :, n0 + t0:n0 + t0 + tw])
                    else:
                        st = xpool.tile([P, DK, tw], dtx(t), tag="xst",
                                        name=f"{nm}_{j}")
                        nc.sync.dma_start(st,
                                          xr[:, :, n0 + t0:n0 + t0 + tw])
                    tiles[j] = st
                dmas.append(mk)
            return tiles.get, dmas

        def proj_feat_group(x_get, q, tw, w_sb, dst, dst0, off=0, f8=False):
            # one ≤512-wide output tile of a feat-major projection from
            # quarter tile q (dst cols dst0+512q+off ...)
            x_sb = x_get(q)
            c0 = dst0 + q * 512 + off
            ps = ps_sc.tile([P, 512], dt.float32, tag="sc", name="ps_pr")
            if f8:
                for c in range(DK // 2):
                    nc.tensor.matmul(ps[:, :tw], lhsT=w_sb[:, c, :, :],
                                     rhs=x_sb[:, c, :, off:off + tw],
                                     start=(c == 0), stop=(c == DK // 2 - 1),
                                     perf_mode=PM)
            else:
                for kc in range(DK):
                    nc.tensor.matmul(ps[:, :tw], lhsT=w_sb[:, kc, :],
                                     rhs=x_sb[:, kc, off:off + tw],
                                     start=(kc == 0), stop=(kc == DK - 1))
            nc.vector.tensor_copy(out=dst[:, c0:c0 + tw], in_=ps[:, :tw])

        def proj_feat_split(x_get, q, tw, w_sb, dst, dst0, f8=False):
            # proj_feat_group split into two kc-halves to halve the PE
            # displacement when dropped into a chunk loop
            cell = {}
            HK = (DK // 2) // 2 if f8 else DK // 2

            def partA():
                cell["ps"] = ps_sc.tile([P, 512], dt.float32, tag="sc",
                                        name="ps_pr")
                ps = cell["ps"]
                if f8:
                    for c in range(HK):
                        nc.tensor.matmul(ps[:, :tw], lhsT=w_sb[:, c, :, :],
                                         rhs=x_get(q)[:, c, :, 0:tw],
                                         start=(c == 0), stop=False,
                                         perf_mode=PM)
                else:
                    for kc in range(HK):
                        nc.tensor.matmul(ps[:, :tw], lhsT=w_sb[:, kc, :],
                                         rhs=x_get(q)[:, kc, 0:tw],
                                         start=(kc == 0), stop=False)

            def partB():
                ps = cell["ps"]
                NK = DK // 2 if f8 else DK
                if f8:
                    for c in range(HK, NK):
                        nc.tensor.matmul(ps[:, :tw], lhsT=w_sb[:, c, :, :],
                                         rhs=x_get(q)[:, c, :, 0:tw],
                                         start=False, stop=(c == NK - 1),
                                         perf_mode=PM)
                else:
                    for kc in range(HK, NK):
                        nc.tensor.matmul(ps[:, :tw], lhsT=w_sb[:, kc, :],
                                         rhs=x_get(q)[:, kc, 0:tw],
                                         start=False, stop=(kc == NK - 1))
                nc.vector.tensor_copy(out=dst[:, dst0 + q * 512:
                                              dst0 + q * 512 + tw],
                                      in_=ps[:, :tw])
            return partA, partB

        def proj_tok_group(x_get, w_sb, b, kcl):
            # one 128-token chunk of the token-major V projection
            g = b * 16 + kcl
            x_sb = x_get(kcl // 4)
            t0 = (kcl % 4) * 128
            ps = ps_sc.tile([P, F], dt.float32, tag="sc", name="ps_v")
            if F8["v"]:
                for c in range(DK // 2):
                    nc.tensor.matmul(ps, lhsT=x_sb[:, c, :, t0:t0 + 128],
                                     rhs=w_sb[:, c, :, :],
                                     start=(c == 0), stop=(c == DK // 2 - 1),
                                     perf_mode=PM)
            else:
                for kc in range(DK):
                    nc.tensor.matmul(ps, lhsT=x_sb[:, kc, t0:t0 + 128],
                                     rhs=w_sb[:, kc, :],
                                     start=(kc == 0), stop=(kc == DK - 1))
            nc.vector.tensor_copy(out=V[:, g, 0:64], in_=ps[:, 0:64])
            nc.vector.tensor_copy(out=V[:, g, 65:129], in_=ps[:, 64:128])

        def tiles_of(w):
            return [(t0, min(512, w - t0)) for t0 in range(0, w, 512)]

        # filler queues: closures emitting one PE work group each, drained
        # into the attn chunk loop's exp-wait gaps. Tails drain only from
        # kcl>=2 so their transposes don't block PE on the qt-boundary
        # normalize chain.
        tail_q: list = []
        bulk_q: list = []

        def drain_one(kcl=2):
            if kcl >= 2 and tail_q:
                tail_q.pop(0)()
            elif bulk_q:
                bulk_q.pop(0)()
            elif kcl >= 2 and tail_q:
                tail_q.pop(0)()

        def qt_tail_items(b, qt, aos, split_dma=False, use_act=False):
            # the Wo tail split into 5 small filler items so it drains into
            # chunk-loop slack instead of stalling ACT at the qt boundary
            q0 = b * S + qt * 512
            cell = {}

            def item0():
                oT = otpool.tile([P, 512], DT_ATTN, tag="oT", name="oT")
                for qs in range(4):
                    tr = ps_wo.tile([P, P], DT_ATTN, tag="pw", name="tr")
                    nc.tensor.transpose(tr, aos[qs], ident)
                    nc.vector.tensor_copy(out=oT[:, qs * 128:(qs + 1) * 128],
                                          in_=tr)
                cell["oT"] = oT
                cell["ost"] = ospool.tile([P, 4, D], DT_OUT, tag="ost",
                                          name="ost")

            def mk_wo(sc4):
                def item():
                    oT, ost = cell["oT"], cell["ost"]
                    for half in range(2):
                        pw = ps_wo.tile([P, 512], dt.float32, tag="pw",
                                        name="pw")
                        nc.tensor.matmul(
                            pw, lhsT=oT[:, sc4 * 128:(sc4 + 1) * 128],
                            rhs=wo_sb[:, half * 512:(half + 1) * 512])
                        dst = ost[:, sc4, half * 512:(half + 1) * 512]
                        if use_act and half == 0:
                            nc.scalar.copy(out=dst, in_=pw)
                        else:
                            nc.vector.tensor_copy(out=dst, in_=pw)
                    gs0 = q0 // 128
                    orr = out_d.rearrange("(g p) n -> p g n", p=P)
                    if split_dma and sc4 == 1:
                        nc.sync.dma_start(orr[:, gs0:gs0 + 2, :],
                                          cell["ost"][:, 0:2, :])
                    if sc4 == 3:
                        if split_dma:
                            nc.sync.dma_start(orr[:, gs0 + 2:gs0 + 4, :],
                                              cell["ost"][:, 2:4, :])
                        else:
                            nc.sync.dma_start(orr[:, gs0:gs0 + 4, :],
                                              cell["ost"])
                return item
            return [item0, mk_wo(0), mk_wo(1), mk_wo(2), mk_wo(3)]

        # ---- schedule ----
        # b0: qt0's chunk loop doubles as the startup ramp — K/V quarter
        # DMAs + projections are embedded so exp starts after ~3 chunk DMAs.
        # b1: x DMAs and proj groups become filler drained through b0's attn.
        # qt Wo-tails are deferred one qt (front of the filler queue).
        def attn_qt(b, qt, embed=None, late=None, last=False):
            late = late or {}
            q0 = b * S + qt * 512
            # start=True only on the FIRST matmul touching each bank: the
            # pending-zero region is the whole 2KB bank, so qs>0 chunk-0
            # matmuls must use start=False (their bytes are zero-filled by
            # qs0's mark; a second start=True would wipe qs0's result)
            av0 = ps_av.tile([P, 4, 65], dt.float32, tag="av0", name="av0")
            av1 = ps_av.tile([P, 4, 65], dt.float32, tag="av1", name="av1")
            pend = None
            for kcl in range(KC[b]):
                if embed is not None:
                    embed(kcl)
                if kcl in late:
                    late[kcl]()
                g = b * 16 + kcl
                k0 = b * S + kcl * 128
                sc2 = ps_sc.tile([P, 2, 512], dt.float32, tag="sc",
                                 name="sc2")
                nc.tensor.matmul(sc2[:, 0, :], lhsT=KT[0:64, k0:k0 + 128],
                                 rhs=QT[0:64, q0:q0 + 512])
                nc.tensor.matmul(sc2[:, 1, :], lhsT=KT[64:128, k0:k0 + 128],
                                 rhs=QT[64:128, q0:q0 + 512])
                at = apool.tile([P, 2, 512], DT_ATTN, tag="at", name="at")
                nc.scalar.activation(at.rearrange("p a n -> p (a n)"),
                                     sc2.rearrange("p a n -> p (a n)"),
                                     Exp, bias=mask_sb[:, g:g + 1],
                                     scale=0.125)
                # AV of the previous chunk runs after this chunk's scores so
                # PE never blocks on the current exp (1-deep pipeline)
                if pend is not None:
                    pend()
                if embed is None and kcl >= 1:
                    drain_one(kcl)

                def mk_av(at=at, g=g, st=(kcl == 0),
                          sp=(kcl == KC[b] - 1)):
                    for qs in range(4):
                        nc.tensor.matmul(
                            av0[:, qs, :],
                            lhsT=at[:, 0, qs * 128:(qs + 1) * 128],
                            rhs=V[:, g, 0:65], start=(st and qs == 0),
                            stop=sp)
                        nc.tensor.matmul(
                            av1[:, qs, :],
                            lhsT=at[:, 1, qs * 128:(qs + 1) * 128],
                            rhs=V[:, g, 65:130], start=(st and qs == 0),
                            stop=sp)
                pend = mk_av
            pend()
            # softmax denominators are per-partition (col 64): normalize
            aoq = aopool.tile([P, 4, P], DT_ATTN, tag="ao", name="aoq")
            for h, av in ((0, av0), (1, av1)):
                rc = rpool.tile([P, 4, 1], dt.float32, tag="rc", name="rc")
                nc.vector.reciprocal(rc, av[:, :, 64:65])
                in0, in1 = bass.broadcast_tensor_aps(av[:, :, 0:64],
                                                     rc[:, :, 0:1])
                nc.vector.tensor_tensor(
                    out=aoq[:, :, h * 64:(h + 1) * 64], in0=in0, in1=in1,
                    op=mybir.AluOpType.mult)
            aos = [aoq[:, qs, :] for qs in range(4)]
            if last:
                for it in qt_tail_items(b, qt, aos, split_dma=True,
                                        use_act=True):
                    it()
            else:
                # defer the Wo tail into the next qt's chunk loop
                tail_q.extend(qt_tail_items(b, qt, aos))

        # --- batch 0 startup ---
        gV0, pV0 = x_chunks(xtv_d, 0, KW[0], "xv0", "v")
        gK0, pK0 = x_chunks(xtk_d, 0, KW[0], "xk0", "k")
        gQ0, pQ0 = x_chunks(xtq_d, 0, S, "xq0", "q")
        nqKV = len(pV0)
        # prologue: DMA order = first-use order. mask is tiny but gates the
        # first exp via ACT program order, so it goes right after wk.
        dve_load(wk_sb, wre(wk_d, "k"), wsh("k"), dtx("k"), "wk")
        mask_raw = const.tile([P, B * 16], dt.float32, tag="mask_raw")
        nc.sync.dma_start(mask_raw, mask_d[:, :])
        nc.scalar.copy(out=mask_sb, in_=mask_raw)
        dve_load(wq_sb, wre(wq_d, "q"), wsh("q"), dtx("q"), "wq")
        pQ0[0]()
        pK0[0]()
        dve_load(wv_sb, wre(wv_d, "v"), wsh("v"), dtx("v"), "wv")
        pV0[0]()
        if nqKV > 1:
            pK0[1]()
            pV0[1]()
        late_consts()
        kt_tiles = tiles_of(KW[0])

        def embed0(kcl):
            if kcl % 4 == 0:
                q = kcl // 4
                if q + 2 < nqKV:
                    pK0[q + 2]()
                    pV0[q + 2]()
                if kcl == 4 or (KC[0] <= 4 and kcl == 0):
                    for t in range(1, 4):
                        pQ0[t]()
                if q > 0 and q < len(kt_tiles):
                    proj_feat_group(gK0, q, kt_tiles[q][1], wk_sb, KT, 0, f8=F8["k"])
            if kcl == 0:
                # minimal path to the first exps: Q t0 and K quarter 0; the
                # V projections wait until kcl 1 (first needed by av(c0))
                proj_feat_group(gQ0, 0, 512, wq_sb, QT, 0, f8=F8["q"])
                proj_feat_group(gK0, 0, min(128, KW[0]), wk_sb, KT, 0,
                                f8=F8["k"])
                if KW[0] > 128:
                    proj_feat_group(gK0, 0, min(KW[0], 512) - 128, wk_sb,
                                    KT, 0, off=128, f8=F8["k"])
                if KC[0] == 1:
                    proj_tok_group(gV0, wv_sb, 0, 0)
            elif kcl == 1:
                for c in range(0, min(3, KC[0])):
                    proj_tok_group(gV0, wv_sb, 0, c)
            elif kcl + 1 < KC[0]:
                proj_tok_group(gV0, wv_sb, 0, kcl + 1)
            if KC[0] <= 8 and kcl == KC[0] - 1:
                for t in range(1, 4):
                    pQ0[t]()

        qA, qB = proj_feat_split(gQ0, 1, 512, wq_sb, QT, 0, f8=F8["q"])
        attn_qt(0, 0, embed=embed0,
                late={min(KC[0] - 2, 8): qA, min(KC[0] - 1, 9): qB})

        # --- batch 1 prefetch as filler (drained through b0 qt1-3) ---
        if B > 1:
            gV1, pV1 = x_chunks(xtv_d, 1, KW[1], "xv1", "v")
            gK1, pK1 = x_chunks(xtk_d, 1, KW[1], "xk1", "k")
            gQ1, pQ1 = x_chunks(xtq_d, 1, S, "xq1", "q")
            items = []
            kt1 = tiles_of(KW[1])
            nq1 = len(pV1)

            def v_pair(c0):
                def it():
                    for c in range(c0, min(c0 + 2, KC[1])):
                        proj_tok_group(gV1, wv_sb, 1, c)
                return it
            for j in range(nq1):
                items.append(pK1[j])
                items.append(pV1[j])
                if j >= 1:
                    jj = j - 1
                    items.append((lambda q, d: lambda: proj_feat_group(
                        gK1, q, d, wk_sb, KT, S, f8=F8["k"]))(jj, kt1[jj][1]))
                    for c0 in range(4 * jj, min(4 * jj + 4, KC[1]), 2):
                        items.append(v_pair(c0))
            items.append((lambda q, d: lambda: proj_feat_group(
                gK1, q, d, wk_sb, KT, S, f8=F8["k"]))(nq1 - 1, kt1[-1][1]))
            for c0 in range(max(0, 4 * (nq1 - 1)), KC[1], 2):
                items.append(v_pair(c0))
            items.append(pQ1[0])
            items.append(pQ1[1])
            items.append((lambda: lambda: proj_feat_group(
                gQ1, 0, 512, wq_sb, QT, S, f8=F8["q"]))())
            bulk_q.extend(items)

        qA, qB = proj_feat_split(gQ0, 2, 512, wq_sb, QT, 0, f8=F8["q"])
        attn_qt(0, 1, late={max(1, KC[0] // 2 - 1): qA, KC[0] // 2: qB})
        qA, qB = proj_feat_split(gQ0, 3, 512, wq_sb, QT, 0, f8=F8["q"])
        attn_qt(0, 2, late={max(1, KC[0] // 2 - 1): qA, KC[0] // 2: qB})
        attn_qt(0, 3)
        # batch-1 attention reads KT/V/QT(b1): flush any un-drained
        # projection work before the first read is emitted
        while bulk_q:
            drain_one(0)
        if B > 1:
            pQ1[2]()
            pQ1[3]()
            mid = max(1, KC[1] // 2 - 1)
            for qt in range(4):
                late = {}
                if qt < 3:
                    qA, qB = proj_feat_split(gQ1, qt + 1, 512, wq_sb, QT, S,
                                             f8=F8["q"])
                    late = {mid: qA, mid + 1: qB}
                attn_qt(1, qt, late=late, last=(qt == 3))
        while tail_q or bulk_q:
            drain_one(2)

    _legalize_waits(nc)
    return nc


def _legalize_waits(nc):
    """This walrus build accepts at most ONE sync-wait command per
    instruction, while Tile emits up to a dozen (e.g. the kernel-tail
    drain). Legalize by splitting: excess waits are hoisted onto
    same-engine Drain instructions inserted immediately before the
    offender — same-engine program order makes this semantically
    identical. Patched module is served via nc.to_json_bytes."""
    import json as _json

    raw = nc.to_json_bytes()
    d = _json.loads(raw)
    template = None
    for fn in d.get("functions", []):
        for blk in fn.get("blocks", []):
            for inst in blk.get("instructions", []):
                if inst.get("opcode") == "Drain":
                    template = inst
                    break
            if template:
                break
        if template:
            break
    assert template is not None, "no Drain template found"

    counter = [0]

    def carrier(engine, wait):
        counter[0] += 1
        c = _json.loads(_json.dumps(template))
        c["name"] = f"I-waitfix-{counter[0]}"
        c["engine"] = engine
        c["sync_info"] = {"on_update": [], "on_wait": [wait]}
        c["ins"] = []
        c["outs"] = []
        return c

    nfix = 0
    for fn in d.get("functions", []):
        for blk in fn.get("blocks", []):
            out = []
            for inst in blk.get("instructions", []):
                si = inst.get("sync_info")
                waits = (si or {}).get("on_wait") or []
                if len(waits) > 1:
                    for w in waits[:-1]:
                        out.append(carrier(inst["engine"], w))
                    si["on_wait"] = [waits[-1]]
                    nfix += 1
                out.append(inst)
            blk["instructions"] = out

    patched = _json.dumps(d).encode()
    nc.to_json_bytes = lambda: patched


def _prep_host(queries, keys, values, Wq, Wk, Wv, Wo, valid_lens, cfg):
    np_map = {"bfloat16": ml_dtypes.bfloat16, "float32": np.float32,
              "float16": np.float16}
    p8 = str(cfg.get("proj8", "0"))
    F8 = {"q": p8 in ("1", "qk", "q"), "k": p8 in ("1", "qk", "q"),
          "v": p8 == "1"}
    np_in = np_map[cfg["dt_in"]]

    def np_x(t):
        return ml_dtypes.float8_e4m3 if F8[t] else np_in
    L = [int(valid_lens[0]), int(valid_lens[1])]
    KC = tuple(min(16, (l + 127) // 128) for l in L)

    def t2(x, t):  # (B,S,D) -> (D, B*S)
        return np.ascontiguousarray(
            np.asarray(x, np.float32).reshape(NT, D).T).astype(np_x(t))

    xtq, xtk, xtv = t2(queries, "q"), t2(keys, "k"), t2(values, "v")
    maskt = np.full((P, B * 16), NEG, np.float32)
    for b in range(B):
        for c in range(16):
            ks = c * 128 + np.arange(P)
            maskt[:, b * 16 + c] = np.where(ks < L[b], 0.0, NEG)

    Wq = np.asarray(Wq, np.float32)
    Wk = np.asarray(Wk, np.float32)
    Wv = np.asarray(Wv, np.float32)
    Wo = np.asarray(Wo, np.float32)
    in_maps = []
    for c in range(N_CORES):
        cs = slice(c * F, (c + 1) * F)
        in_maps.append({
            "xtq": xtq, "xtk": xtk, "xtv": xtv,
            "wq": np.ascontiguousarray(Wq[:, cs]).astype(np_x("q")),
            "wk": np.ascontiguousarray(Wk[:, cs]).astype(np_x("k")),
            "wv": np.ascontiguousarray(Wv[:, cs]).astype(np_x("v")),
            "wo": np.ascontiguousarray(Wo[cs, :]).astype(np_in),
            "maskt": maskt,
        })
    return KC, in_maps


DEFAULT_CFG = {"dt_in": "float16", "dt_attn": "float16", "dt_out": "float16",
               "proj8": "0"}

LAST_RESULTS = None


def kernel(queries, keys, values, Wq, Wk, Wv, Wo, valid_lens):
    global LAST_RESULTS
    from concourse.bass_utils import run_bass_kernel_spmd

    cfg = dict(DEFAULT_CFG)
    if os.environ.get("MHA_CFG"):
        for kv in os.environ["MHA_CFG"].split(","):
            k, v = kv.split("=")
            cfg[k] = v

    KC, in_maps = _prep_host(queries, keys, values, Wq, Wk, Wv, Wo,
                             valid_lens, cfg)
    key = (KC, tuple(sorted(cfg.items())))
    if key not in _CACHE:
        _CACHE[key] = _build_program(KC, cfg)
    nc = _CACHE[key]

    trace = bool(os.environ.get("MHA_TRACE"))
    res = run_bass_kernel_spmd(nc, in_maps, core_ids=list(range(N_CORES)),
                               trace=trace)
    LAST_RESULTS = res
    acc = np.zeros((NT, D), np.float32)
    for r in res.results:
        acc += np.asarray(r["out_part"], np.float32)
    return acc.reshape(B, S, D)


# revision 32
# speedup vs baseline: 1.3568x; 1.0074x over previous
"""Multi-head attention (16 heads, D=1024, B=2, S=2048) on 8 TRN2 NeuronCores.

Sharding: tensor-parallel over heads. Each core owns 2 heads (128 features):
W_q/k/v column-sliced, W_o row-sliced; partial outputs summed on host.

Device dataflow (per core):
  QT[f,s], KT[f,s] = W^T x^T   (feat-major projections, contraction on parts)
  V[k,f]           = x W       (token-major projection, k on partitions)
  scores^T[k,q] = KT_h^T . QT_h  per 128-k chunk, both heads -> one psum pair
  attn^T = exp(scores*0.125 + mask_bias[k])  (ACT, psum->SBUF fp16)
  av[q, 65] += attn_chunk_h^T . [V_h | 1]    (N=65 matmuls, psum accum over k;
        col 64 = softmax denominator, per-partition in q!)
  recip = 1/rowsum (DVE), attn_out[q,f] = av * recip  (tensor_scalar per-part)
  outT[f,q] via PE transpose;  out_part[s,:] = outT^T . Wo  -> fp16, host sums

Key-padding mask: k-chunks beyond valid_len are skipped (program specialized
on valid_lens); boundary chunk masked via -1e6 exp bias (underflows to 0).

Build order = schedule: engines run in program order, so proj(b+1) groups and
the qt Wo-tails are interleaved into the attn chunk loop as PE filler to keep
ACT (the exp bottleneck) saturated.

cfg proj8=1: x and W_q/k/v in fp8e4 with DoubleRow matmuls (2x PE rate,
half the x DMA bytes). Layout is a pure AP rearrange: contraction index
d = c*256 + two*128 + p for both weights and x, so the sum is unchanged.
"""

import math
import os

import ml_dtypes
import numpy as np

B = 2
S = 2048
D = 1024
NT = B * S          # 4096 rows, b-major
F = 128             # features per core (2 heads x 64)
DH = 64
P = 128
DK = D // P         # 8 contraction chunks for projections
N_CORES = 8
NEG = -1e6

_CACHE: dict = {}


def _build_program(KC: tuple[int, int], cfg: dict):
    import concourse.bass as bass
    import concourse.tile as tile
    from concourse import mybir
    from concourse.masks import make_identity

    dt = mybir.dt
    DT_IN = getattr(dt, cfg["dt_in"])        # W_o / non-fp8 operands
    DT_ATTN = getattr(dt, cfg["dt_attn"])    # attn / V / QT / KT storage
    DT_OUT = getattr(dt, cfg["dt_out"])      # partial output in HBM
    p8 = str(cfg.get("proj8", "0"))
    F8 = {"q": p8 in ("1", "qk", "q"), "k": p8 in ("1", "qk", "q"),
          "v": p8 == "1"}
    def dtx(t):
        return dt.float8e4 if F8[t] else DT_IN
    PM = mybir.MatmulPerfMode.DoubleRow
    Exp = mybir.ActivationFunctionType.Exp

    nc = bass.Bass("TRN2")

    xtq_d = nc.dram_tensor("xtq", [D, NT], dtx("q"), kind="ExternalInput")
    xtk_d = nc.dram_tensor("xtk", [D, NT], dtx("k"), kind="ExternalInput")
    xtv_d = nc.dram_tensor("xtv", [D, NT], dtx("v"), kind="ExternalInput")
    wq_d = nc.dram_tensor("wq", [D, F], dtx("q"), kind="ExternalInput")
    wk_d = nc.dram_tensor("wk", [D, F], dtx("k"), kind="ExternalInput")
    wv_d = nc.dram_tensor("wv", [D, F], dtx("v"), kind="ExternalInput")
    wo_d = nc.dram_tensor("wo", [F, D], DT_IN, kind="ExternalInput")
    mask_d = nc.dram_tensor("maskt", [P, B * 16], dt.float32, kind="ExternalInput")
    out_d = nc.dram_tensor("out_part", [NT, D], DT_OUT, kind="ExternalOutput")

    KW = [KC[0] * 128, KC[1] * 128]   # K/V token count per batch

    from contextlib import ExitStack

    with tile.TileContext(nc) as tc, ExitStack() as ctx:
        const = ctx.enter_context(tc.tile_pool(name="const", bufs=1))
        xpool = ctx.enter_context(tc.tile_pool(name="xpool", bufs=12))
        apool = ctx.enter_context(tc.tile_pool(name="apool", bufs=4))
        aopool = ctx.enter_context(tc.tile_pool(name="aopool", bufs=5))
        otpool = ctx.enter_context(tc.tile_pool(name="otpool", bufs=2))
        ospool = ctx.enter_context(tc.tile_pool(name="ospool", bufs=2))
        rpool = ctx.enter_context(tc.tile_pool(name="rpool", bufs=4))
        ps_sc = ctx.enter_context(
            tc.tile_pool(name="ps_sc", bufs=2, space="PSUM"))
        ps_av = ctx.enter_context(
            tc.tile_pool(name="ps_av", bufs=1, space="PSUM"))
        ps_wo = ctx.enter_context(
            tc.tile_pool(name="ps_wo", bufs=2, space="PSUM"))

        # ---- constants ----
        # Matmult instructions tolerate only ONE sync-wait; weight/identity
        # loads bounce DRAM -> raw tile -> DVE copy so matmul deps merge.
        def dve_load(dst, src_ap, raw_shape, raw_dtype, nm):
            raw = const.tile(list(raw_shape), raw_dtype, tag=f"{nm}_raw",
                             name=f"{nm}_raw")
            nc.sync.dma_start(raw, src_ap)
            nc.vector.tensor_copy(out=dst, in_=raw)

        def wsh(t):
            return [P, DK // 2, 2, F] if F8[t] else [P, DK, F]

        def wre(wd, t):
            if F8[t]:
                return wd.rearrange("(c two p) f -> p c two f", p=P, two=2)
            return wd.rearrange("(kc p) f -> p kc f", p=P)
        wq_sb = const.tile(wsh("q"), dtx("q"), tag="wq")
        wk_sb = const.tile(wsh("k"), dtx("k"), tag="wk")
        wv_sb = const.tile(wsh("v"), dtx("v"), tag="wv")
        mask_sb = const.tile([P, B * 16], dt.float32, tag="mask")
        # wo / identity are needed only by the first qt tail (~30us in);
        # emitted after the startup x DMAs so they don't delay them
        wo_sb = const.tile([F, D], DT_IN, tag="wo")
        ident = const.tile([P, P], DT_ATTN, tag="ident")

        def late_consts():
            dve_load(wo_sb, wo_d[:, :], [F, D], DT_IN, "wo")
            ident_g = const.tile([P, P], DT_ATTN, tag="ident_g")
            make_identity(nc, ident_g)
            nc.vector.tensor_copy(out=ident, in_=ident_g)

        # PE warmup: a few junk matmuls anchor the p-state ramp so the
        # first projections run at full clock (scratch psum, never read)
        warm = const.tile([P, 512], DT_ATTN, tag="warm")
        nc.vector.memset(warm, 1.0)
        for _ in range(8):
            wps = ps_sc.tile([P, 512], dt.float32, tag="sc", name="wps")
            nc.tensor.matmul(wps, lhsT=warm[:, 0:128], rhs=warm)
        # drain holds the PE sequencer until the warmup completes, so real
        # matmuls dispatch with the p-state ramp already past 3us (the cost
        # model prices matmuls at dispatch time)
        nc.tensor.drain()

        QT = const.tile([P, NT], DT_ATTN, tag="QT")
        KT = const.tile([P, NT], DT_ATTN, tag="KT")
        # V natural layout (k on partitions) per 128-k chunk:
        # cols 0:64 = head0, col 64 = ones, cols 65:129 = head1, col 129 = ones
        V = const.tile([P, B * 16, 130], DT_ATTN, tag="V")
        nc.vector.memset(V[:, :, 64:65], 1.0)
        nc.vector.memset(V[:, :, 129:130], 1.0)

        # ---- x staging: one tile per 512-token chunk, created lazily in
        # its DMA closure so ring order == issue order; projections look the
        # quarter tile up at emission time (always after its DMA) ----
        def x_chunks(xd, b, w, nm, t):
            f8 = F8[t]
            if f8:
                xr = xd.rearrange("(c two p) n -> p c two n", p=P, two=2)
            else:
                xr = xd.rearrange("(kc p) n -> p kc n", p=P)
            n0 = b * S
            tiles = {}
            dmas = []
            for j, t0 in enumerate(range(0, w, 512)):
                tw = min(512, w - t0)

                def mk(j=j, t0=t0, tw=tw):
                    if f8:
                        st = xpool.tile([P, DK // 2, 2, tw], dtx(t),
                                        tag="xst", name=f"{nm}_{j}")
                        nc.sync.dma_start(st,
                                          xr[:, :, :, n0 + t0:n0 + t0 + tw])
                    else:
                        st = xpool.tile([P, DK, tw], dtx(t), tag="xst",
                                        name=f"{nm}_{j}")
                        nc.sync.dma_start(st,
                                          xr[:, :, n0 + t0:n0 + t0 + tw])
                    tiles[j] = st
                dmas.append(mk)
            return tiles.get, dmas

        def proj_feat_group(x_get, q, tw, w_sb, dst, dst0, off=0, f8=False):
            # one ≤512-wide output tile of a feat-major projection from
            # quarter tile q (dst cols dst0+512q+off ...)
            x_sb = x_get(q)
            c0 = dst0 + q * 512 + off
            ps = ps_sc.tile([P, 512], dt.float32, tag="sc", name="ps_pr")
            if f8:
                for c in range(DK // 2):
                    nc.tensor.matmul(ps[:, :tw], lhsT=w_sb[:, c, :, :],
                                     rhs=x_sb[:, c, :, off:off + tw],
                                     start=(c == 0), stop=(c == DK // 2 - 1),
                                     perf_mode=PM)
            else:
                for kc in range(DK):
                    nc.tensor.matmul(ps[:, :tw], lhsT=w_sb[:, kc, :],
                                     rhs=x_sb[:, kc, off:off + tw],
                                     start=(kc == 0), stop=(kc == DK - 1))
            nc.vector.tensor_copy(out=dst[:, c0:c0 + tw], in_=ps[:, :tw])

        def proj_feat_split(x_get, q, tw, w_sb, dst, dst0, f8=False):
            # proj_feat_group split into two kc-halves to halve the PE
            # displacement when dropped into a chunk loop
            cell = {}
            HK = (DK // 2) // 2 if f8 else DK // 2

            def partA():
                cell["ps"] = ps_sc.tile([P, 512], dt.float32, tag="sc",
                                        name="ps_pr")
                ps = cell["ps"]
                if f8:
                    for c in range(HK):
                        nc.tensor.matmul(ps[:, :tw], lhsT=w_sb[:, c, :, :],
                                         rhs=x_get(q)[:, c, :, 0:tw],
                                         start=(c == 0), stop=False,
                                         perf_mode=PM)
                else:
                    for kc in range(HK):
                        nc.tensor.matmul(ps[:, :tw], lhsT=w_sb[:, kc, :],
                                         rhs=x_get(q)[:, kc, 0:tw],
                                         start=(kc == 0), stop=False)

            def partB():
                ps = cell["ps"]
                NK = DK // 2 if f8 else DK
                if f8:
                    for c in range(HK, NK):
                        nc.tensor.matmul(ps[:, :tw], lhsT=w_sb[:, c, :, :],
                                         rhs=x_get(q)[:, c, :, 0:tw],
                                         start=False, stop=(c == NK - 1),
                                         perf_mode=PM)
                else:
                    for kc in range(HK, NK):
                        nc.tensor.matmul(ps[:, :tw], lhsT=w_sb[:, kc, :],
                                         rhs=x_get(q)[:, kc, 0:tw],
                                         start=False, stop=(kc == NK - 1))
                nc.vector.tensor_copy(out=dst[:, dst0 + q * 512:
                                              dst0 + q * 512 + tw],
                                      in_=ps[:, :tw])
            return partA, partB

        def proj_tok_group(x_get, w_sb, b, kcl):
            # one 128-token chunk of the token-major V projection
            g = b * 16 + kcl
            x_sb = x_get(kcl // 4)
            t0 = (kcl % 4) * 128
            ps = ps_sc.tile([P, F], dt.float32, tag="sc", name="ps_v")
            if F8["v"]:
                for c in range(DK // 2):
                    nc.tensor.matmul(ps, lhsT=x_sb[:, c, :, t0:t0 + 128],
                                     rhs=w_sb[:, c, :, :],
                                     start=(c == 0), stop=(c == DK // 2 - 1),
                                     perf_mode=PM)
            else:
                for kc in range(DK):
                    nc.tensor.matmul(ps, lhsT=x_sb[:, kc, t0:t0 + 128],
                                     rhs=w_sb[:, kc, :],
                                     start=(kc == 0), stop=(kc == DK - 1))
            nc.vector.tensor_copy(out=V[:, g, 0:64], in_=ps[:, 0:64])
            nc.vector.tensor_copy(out=V[:, g, 65:129], in_=ps[:, 64:128])

        def tiles_of(w):
            return [(t0, min(512, w - t0)) for t0 in range(0, w, 512)]

        # filler queues: closures emitting one PE work group each, drained
        # into the attn chunk loop's exp-wait gaps. Tails drain only from
        # kcl>=2 so their transposes don't block PE on the qt-boundary
        # normalize chain.
        tail_q: list = []
        bulk_q: list = []

        def drain_one(kcl=2):
            if kcl >= 2 and tail_q:
                tail_q.pop(0)()
            elif bulk_q:
                bulk_q.pop(0)()
            elif kcl >= 2 and tail_q:
                tail_q.pop(0)()

        def qt_tail_items(b, qt, aos, split_dma=False, use_act=False,
                          use_sc=False):
            # the Wo tail split into 5 small filler items so it drains into
            # chunk-loop slack instead of stalling ACT at the qt boundary
            q0 = b * S + qt * 512
            cell = {}

            def item0():
                oT = otpool.tile([P, 512], DT_ATTN, tag="oT", name="oT")
                for qs in range(4):
                    tr = ps_wo.tile([P, P], DT_ATTN, tag="pw", name="tr")
                    nc.tensor.transpose(tr, aos[qs], ident)
                    nc.vector.tensor_copy(out=oT[:, qs * 128:(qs + 1) * 128],
                                          in_=tr)
                cell["oT"] = oT
                cell["ost"] = ospool.tile([P, 4, D], DT_OUT, tag="ost",
                                          name="ost")

            def mk_wo(sc4):
                def item():
                    oT, ost = cell["oT"], cell["ost"]
                    for half in range(2):
                        if use_sc:
                            pw = ps_sc.tile([P, 512], dt.float32, tag="sc",
                                            name="pw")
                        else:
                            pw = ps_wo.tile([P, 512], dt.float32, tag="pw",
                                            name="pw")
                        nc.tensor.matmul(
                            pw, lhsT=oT[:, sc4 * 128:(sc4 + 1) * 128],
                            rhs=wo_sb[:, half * 512:(half + 1) * 512])
                        dst = ost[:, sc4, half * 512:(half + 1) * 512]
                        if use_act and half == 0:
                            nc.scalar.copy(out=dst, in_=pw)
                        else:
                            nc.vector.tensor_copy(out=dst, in_=pw)
                    gs0 = q0 // 128
                    orr = out_d.rearrange("(g p) n -> p g n", p=P)
                    if split_dma:
                        nc.sync.dma_start(
                            orr[:, gs0 + sc4:gs0 + sc4 + 1, :],
                            cell["ost"][:, sc4:sc4 + 1, :])
                    elif sc4 == 3:
                        nc.sync.dma_start(orr[:, gs0:gs0 + 4, :],
                                          cell["ost"])
                return item
            return [item0, mk_wo(0), mk_wo(1), mk_wo(2), mk_wo(3)]

        # ---- schedule ----
        # b0: qt0's chunk loop doubles as the startup ramp — K/V quarter
        # DMAs + projections are embedded so exp starts after ~3 chunk DMAs.
        # b1: x DMAs and proj groups become filler drained through b0's attn.
        # qt Wo-tails are deferred one qt (front of the filler queue).
        def attn_qt(b, qt, embed=None, late=None, last=False):
            late = late or {}
            q0 = b * S + qt * 512
            # start=True only on the FIRST matmul touching each bank: the
            # pending-zero region is the whole 2KB bank, so qs>0 chunk-0
            # matmuls must use start=False (their bytes are zero-filled by
            # qs0's mark; a second start=True would wipe qs0's result)
            av0 = ps_av.tile([P, 4, 65], dt.float32, tag="av0", name="av0")
            av1 = ps_av.tile([P, 4, 65], dt.float32, tag="av1", name="av1")
            pend = None
            for kcl in range(KC[b]):
                if embed is not None:
                    embed(kcl)
                if kcl in late:
                    late[kcl]()
                g = b * 16 + kcl
                k0 = b * S + kcl * 128
                sc2 = ps_sc.tile([P, 2, 512], dt.float32, tag="sc",
                                 name="sc2")
                nc.tensor.matmul(sc2[:, 0, :], lhsT=KT[0:64, k0:k0 + 128],
                                 rhs=QT[0:64, q0:q0 + 512])
                nc.tensor.matmul(sc2[:, 1, :], lhsT=KT[64:128, k0:k0 + 128],
                                 rhs=QT[64:128, q0:q0 + 512])
                at = apool.tile([P, 2, 512], DT_ATTN, tag="at", name="at")
                nc.scalar.activation(at.rearrange("p a n -> p (a n)"),
                                     sc2.rearrange("p a n -> p (a n)"),
                                     Exp, bias=mask_sb[:, g:g + 1],
                                     scale=0.125)
                # AV of the previous chunk runs after this chunk's scores so
                # PE never blocks on the current exp (1-deep pipeline)
                if pend is not None:
                    pend()
                if embed is None and kcl >= 1:
                    drain_one(kcl)

                def mk_av(at=at, g=g, st=(kcl == 0),
                          sp=(kcl == KC[b] - 1)):
                    for qs in range(4):
                        nc.tensor.matmul(
                            av0[:, qs, :],
                            lhsT=at[:, 0, qs * 128:(qs + 1) * 128],
                            rhs=V[:, g, 0:65], start=(st and qs == 0),
                            stop=sp)
                        nc.tensor.matmul(
                            av1[:, qs, :],
                            lhsT=at[:, 1, qs * 128:(qs + 1) * 128],
                            rhs=V[:, g, 65:130], start=(st and qs == 0),
                            stop=sp)
                pend = mk_av
            pend()
            # softmax denominators are per-partition (col 64): normalize
            aoq = aopool.tile([P, 4, P], DT_ATTN, tag="ao", name="aoq")
            for h, av in ((0, av0), (1, av1)):
                rc = rpool.tile([P, 4, 1], dt.float32, tag="rc", name="rc")
                nc.vector.reciprocal(rc, av[:, :, 64:65])
                in0, in1 = bass.broadcast_tensor_aps(av[:, :, 0:64],
                                                     rc[:, :, 0:1])
                nc.vector.tensor_tensor(
                    out=aoq[:, :, h * 64:(h + 1) * 64], in0=in0, in1=in1,
                    op=mybir.AluOpType.mult)
            aos = [aoq[:, qs, :] for qs in range(4)]
            if last:
                for it in qt_tail_items(b, qt, aos, split_dma=True,
                                        use_act=True, use_sc=True):
                    it()
            else:
                # defer the Wo tail into the next qt's chunk loop
                tail_q.extend(qt_tail_items(b, qt, aos))

        # --- batch 0 startup ---
        gV0, pV0 = x_chunks(xtv_d, 0, KW[0], "xv0", "v")
        gK0, pK0 = x_chunks(xtk_d, 0, KW[0], "xk0", "k")
        gQ0, pQ0 = x_chunks(xtq_d, 0, S, "xq0", "q")
        nqKV = len(pV0)
        # prologue: DMA order = first-use order. mask is tiny but gates the
        # first exp via ACT program order, so it goes right after wk.
        dve_load(wk_sb, wre(wk_d, "k"), wsh("k"), dtx("k"), "wk")
        mask_raw = const.tile([P, B * 16], dt.float32, tag="mask_raw")
        nc.sync.dma_start(mask_raw, mask_d[:, :])
        nc.scalar.copy(out=mask_sb, in_=mask_raw)
        dve_load(wq_sb, wre(wq_d, "q"), wsh("q"), dtx("q"), "wq")
        pQ0[0]()
        pK0[0]()
        dve_load(wv_sb, wre(wv_d, "v"), wsh("v"), dtx("v"), "wv")
        pV0[0]()
        if nqKV > 1:
            pK0[1]()
            pV0[1]()
        late_consts()
        kt_tiles = tiles_of(KW[0])

        def embed0(kcl):
            if kcl % 4 == 0:
                q = kcl // 4
                if q + 2 < nqKV:
                    pK0[q + 2]()
                    pV0[q + 2]()
                if kcl == 4 or (KC[0] <= 4 and kcl == 0):
                    for t in range(1, 4):
                        pQ0[t]()
                if q > 0 and q < len(kt_tiles):
                    proj_feat_group(gK0, q, kt_tiles[q][1], wk_sb, KT, 0, f8=F8["k"])
            if kcl == 0:
                # minimal path to the first exps: Q t0 and K quarter 0; the
                # V projections wait until kcl 1 (first needed by av(c0))
                proj_feat_group(gQ0, 0, 512, wq_sb, QT, 0, f8=F8["q"])
                proj_feat_group(gK0, 0, min(128, KW[0]), wk_sb, KT, 0,
                                f8=F8["k"])
                if KW[0] > 128:
                    proj_feat_group(gK0, 0, min(KW[0], 512) - 128, wk_sb,
                                    KT, 0, off=128, f8=F8["k"])
                if KC[0] == 1:
                    proj_tok_group(gV0, wv_sb, 0, 0)
            elif kcl == 1:
                for c in range(0, min(3, KC[0])):
                    proj_tok_group(gV0, wv_sb, 0, c)
            elif kcl + 1 < KC[0]:
                proj_tok_group(gV0, wv_sb, 0, kcl + 1)
            if KC[0] <= 8 and kcl == KC[0] - 1:
                for t in range(1, 4):
                    pQ0[t]()

        qA, qB = proj_feat_split(gQ0, 1, 512, wq_sb, QT, 0, f8=F8["q"])
        attn_qt(0, 0, embed=embed0,
                late={min(KC[0] - 2, 8): qA, min(KC[0] - 1, 9): qB})

        # --- batch 1 prefetch as filler (drained through b0 qt1-3) ---
        if B > 1:
            gV1, pV1 = x_chunks(xtv_d, 1, KW[1], "xv1", "v")
            gK1, pK1 = x_chunks(xtk_d, 1, KW[1], "xk1", "k")
            gQ1, pQ1 = x_chunks(xtq_d, 1, S, "xq1", "q")
            items = []
            kt1 = tiles_of(KW[1])
            nq1 = len(pV1)

            def v_pair(c0):
                def it():
                    for c in range(c0, min(c0 + 2, KC[1])):
                        proj_tok_group(gV1, wv_sb, 1, c)
                return it
            for j in range(nq1):
                items.append(pK1[j])
                items.append(pV1[j])
                if j >= 1:
                    jj = j - 1
                    items.append((lambda q, d: lambda: proj_feat_group(
                        gK1, q, d, wk_sb, KT, S, f8=F8["k"]))(jj, kt1[jj][1]))
                    for c0 in range(4 * jj, min(4 * jj + 4, KC[1]), 2):
                        items.append(v_pair(c0))
            items.append((lambda q, d: lambda: proj_feat_group(
                gK1, q, d, wk_sb, KT, S, f8=F8["k"]))(nq1 - 1, kt1[-1][1]))
            for c0 in range(max(0, 4 * (nq1 - 1)), KC[1], 2):
                items.append(v_pair(c0))
            items.append(pQ1[0])
            items.append(pQ1[1])
            items.append((lambda: lambda: proj_feat_group(
                gQ1, 0, 512, wq_sb, QT, S, f8=F8["q"]))())
            bulk_q.extend(items)

        qA, qB = proj_feat_split(gQ0, 2, 512, wq_sb, QT, 0, f8=F8["q"])
        attn_qt(0, 1, late={max(1, KC[0] // 2 - 1): qA, KC[0] // 2: qB})
        qA, qB = proj_feat_split(gQ0, 3, 512, wq_sb, QT, 0, f8=F8["q"])
        attn_qt(0, 2, late={max(1, KC[0] // 2 - 1): qA, KC[0] // 2: qB})
        attn_qt(0, 3)
        # batch-1 attention reads KT/V/QT(b1): flush any un-drained
        # projection work before the first read is emitted
        while bulk_q:
            drain_one(0)
        if B > 1:
            pQ1[2]()
            pQ1[3]()
            mid = max(1, KC[1] // 2 - 1)
            for qt in range(4):
                late = {}
                if qt < 3:
                    qA, qB = proj_feat_split(gQ1, qt + 1, 512, wq_sb, QT, S,
                                             f8=F8["q"])
                    late = {mid: qA, mid + 1: qB}
                attn_qt(1, qt, late=late, last=(qt == 3))
        while tail_q or bulk_q:
            drain_one(2)

    _legalize_waits(nc)
    return nc


def _legalize_waits(nc):
    """This walrus build accepts at most ONE sync-wait command per
    instruction, while Tile emits up to a dozen (e.g. the kernel-tail
    drain). Legalize by splitting: excess waits are hoisted onto
    same-engine Drain instructions inserted immediately before the
    offender — same-engine program order makes this semantically
    identical. Patched module is served via nc.to_json_bytes."""
    import json as _json

    raw = nc.to_json_bytes()
    d = _json.loads(raw)
    template = None
    for fn in d.get("functions", []):
        for blk in fn.get("blocks", []):
            for inst in blk.get("instructions", []):
                if inst.get("opcode") == "Drain":
                    template = inst
                    break
            if template:
                break
        if template:
            break
    assert template is not None, "no Drain template found"

    counter = [0]

    def carrier(engine, wait):
        counter[0] += 1
        c = _json.loads(_json.dumps(template))
        c["name"] = f"I-waitfix-{counter[0]}"
        c["engine"] = engine
        c["sync_info"] = {"on_update": [], "on_wait": [wait]}
        c["ins"] = []
        c["outs"] = []
        return c

    nfix = 0
    for fn in d.get("functions", []):
        for blk in fn.get("blocks", []):
            out = []
            for inst in blk.get("instructions", []):
                si = inst.get("sync_info")
                waits = (si or {}).get("on_wait") or []
                if len(waits) > 1:
                    for w in waits[:-1]:
                        out.append(carrier(inst["engine"], w))
                    si["on_wait"] = [waits[-1]]
                    nfix += 1
                out.append(inst)
            blk["instructions"] = out

    patched = _json.dumps(d).encode()
    nc.to_json_bytes = lambda: patched


def _prep_host(queries, keys, values, Wq, Wk, Wv, Wo, valid_lens, cfg):
    np_map = {"bfloat16": ml_dtypes.bfloat16, "float32": np.float32,
              "float16": np.float16}
    p8 = str(cfg.get("proj8", "0"))
    F8 = {"q": p8 in ("1", "qk", "q"), "k": p8 in ("1", "qk", "q"),
          "v": p8 == "1"}
    np_in = np_map[cfg["dt_in"]]

    def np_x(t):
        return ml_dtypes.float8_e4m3 if F8[t] else np_in
    L = [int(valid_lens[0]), int(valid_lens[1])]
    KC = tuple(min(16, (l + 127) // 128) for l in L)

    def t2(x, t):  # (B,S,D) -> (D, B*S)
        return np.ascontiguousarray(
            np.asarray(x, np.float32).reshape(NT, D).T).astype(np_x(t))

    xtq, xtk, xtv = t2(queries, "q"), t2(keys, "k"), t2(values, "v")
    maskt = np.full((P, B * 16), NEG, np.float32)
    for b in range(B):
        for c in range(16):
            ks = c * 128 + np.arange(P)
            maskt[:, b * 16 + c] = np.where(ks < L[b], 0.0, NEG)

    Wq = np.asarray(Wq, np.float32)
    Wk = np.asarray(Wk, np.float32)
    Wv = np.asarray(Wv, np.float32)
    Wo = np.asarray(Wo, np.float32)
    in_maps = []
    for c in range(N_CORES):
        cs = slice(c * F, (c + 1) * F)
        in_maps.append({
            "xtq": xtq, "xtk": xtk, "xtv": xtv,
            "wq": np.ascontiguousarray(Wq[:, cs]).astype(np_x("q")),
            "wk": np.ascontiguousarray(Wk[:, cs]).astype(np_x("k")),
            "wv": np.ascontiguousarray(Wv[:, cs]).astype(np_x("v")),
            "wo": np.ascontiguousarray(Wo[cs, :]).astype(np_in),
            "maskt": maskt,
        })
    return KC, in_maps


DEFAULT_CFG = {"dt_in": "float16", "dt_attn": "float16", "dt_out": "float16",
               "proj8": "0"}

LAST_RESULTS = None


def kernel(queries, keys, values, Wq, Wk, Wv, Wo, valid_lens):
    global LAST_RESULTS
    from concourse.bass_utils import run_bass_kernel_spmd

    cfg = dict(DEFAULT_CFG)
    if os.environ.get("MHA_CFG"):
        for kv in os.environ["MHA_CFG"].split(","):
            k, v = kv.split("=")
            cfg[k] = v

    KC, in_maps = _prep_host(queries, keys, values, Wq, Wk, Wv, Wo,
                             valid_lens, cfg)
    key = (KC, tuple(sorted(cfg.items())))
    if key not in _CACHE:
        _CACHE[key] = _build_program(KC, cfg)
    nc = _CACHE[key]

    trace = bool(os.environ.get("MHA_TRACE"))
    res = run_bass_kernel_spmd(nc, in_maps, core_ids=list(range(N_CORES)),
                               trace=trace)
    LAST_RESULTS = res
    acc = np.zeros((NT, D), np.float32)
    for r in res.results:
        acc += np.asarray(r["out_part"], np.float32)
    return acc.reshape(B, S, D)
